# revision 1
# baseline (speedup 1.0000x reference)
"""Trainium2 Bass kernel for a 4-step differentiable recurrent net forward pass.

Reference computation (B=8192, NI=512, NH=2048, NO=512, 4 steps):
    activs = 0; outputs = 0
    repeat 4x:  pre = hr * (x @ Wih.T + activs @ Whh.T + outputs @ Woh.T) + hb
                activs = per_neuron_act(pre)        # tanh/sigmoid/relu by i%3
    out = sigmoid(or * (x @ Wio.T + outputs @ Woo.T + activs @ Who.T) + ob)

`outputs` is never written inside the loop, so the Woh/Woo terms vanish and
the x-projection P = hr*(x@Wih.T)+hb is loop-invariant (computed once).

Strategy: data-parallel on batch across 8 cores (1024 rows each). On-core
everything is feature-major (features on SBUF partitions, batch on the free
axis), so each matmul is W_tile.T @ X^T with stationary bf16 weights.
Host-side prep: hidden neurons are permuted so the three activation groups
are contiguous, hr/or are folded into the weight matrices, weights are
packed so each loads as one large contiguous DMA, and hb/ob are applied as
per-partition bias APs. Compute is bf16 with f32 PSUM accumulation.
"""

import os

import numpy as np
import ml_dtypes

import concourse.bass as bass
import concourse.tile as tile
from concourse import bacc, mybir
from concourse.bass_utils import run_bass_kernel_spmd

B, NI, NH, NO = 8192, 512, 2048, 512
N_STEPS = 4
N_CORES = 8
BL = B // N_CORES          # batch rows per core
CH = 512                   # batch chunk (one PSUM bank of fp32)
NCH = BL // CH             # 2 chunks per core
KI = NI // 128             # 4 k-tiles over inputs
KH = NH // 128             # 16 k/m-tiles over hidden
KO = NO // 128             # 4 m-tiles over outputs

BF16 = mybir.dt.bfloat16
F32 = mybir.dt.float32
AF = mybir.ActivationFunctionType

# hidden neurons regrouped as [all tanh | all sigmoid | all relu]
_idx = np.arange(NH)
PERM = np.concatenate([_idx[_idx % 3 == 0], _idx[_idx % 3 == 1], _idx[_idx % 3 == 2]])
_B1 = int((_idx % 3 == 0).sum())           # 683
_B2 = _B1 + int((_idx % 3 == 1).sum())     # 1366

# per m-tile: the single activation function, or None for the two mixed tiles
_TILE_FUNC = []
for _m in range(KH):
    _lo, _hi = _m * 128, (_m + 1) * 128
    _fs = set()
    for _f, _a, _b in ((AF.Tanh, 0, _B1), (AF.Sigmoid, _B1, _B2), (AF.Relu, _B2, NH)):
        if max(_lo, _a) < min(_hi, _b):
            _fs.add(_f)
    _TILE_FUNC.append(_fs.pop() if len(_fs) == 1 else None)

# mixed tiles: (major_func applied everywhere, minor_func, mask column block)
# partition sub-ranges must be 32-aligned on TRN2, so the minority strip is
# fixed up with a full-tile ACT + copy_predicated against a {0,1} mask
_BOUNDARY = {
    _B1 // 128: (AF.Sigmoid, AF.Tanh, 0),    # tile 5: parts < 43 are tanh
    _B2 // 128: (AF.Sigmoid, AF.Relu, 1),    # tile 10: parts >= 86 are relu
}


def _emit_hidden_act(nc, ps, blk, a_new, tmp_pool, bmask_t, bias=None):
    """Evict a 4-bank PSUM block through the grouped activations into a_new.

    ps:    PSUM AP (128, 4*CH) holding m-tiles blk*4..blk*4+3 (one per bank)
    a_new: SBUF AP (128, KH*CH) bf16, m-tile m lives at [:, m*CH:(m+1)*CH]
    bias:  optional (128, KH) f32 SBUF tile of per-partition biases; forces
           per-tile ACTs (used on step 1, where PSUM lacks the hidden bias)
    """
    mloc = 0
    while mloc < 4:
        m = blk * 4 + mloc
        bias_ap = bias[:, m:m + 1] if bias is not None else 0.0
        if m in _BOUNDARY:
            major, minor, mb = _BOUNDARY[m]
            nc.scalar.activation(
                a_new[:, m * CH:(m + 1) * CH],
                ps[:, mloc * CH:(mloc + 1) * CH], major, bias=bias_ap)
            t = tmp_pool.tile([128, CH], BF16, tag="btmp", bufs=2, name="btmp")
            nc.scalar.activation(t[:], ps[:, mloc * CH:(mloc + 1) * CH], minor,
                                 bias=bias_ap)
            nc.vector.copy_predicated(
                a_new[:, m * CH:(m + 1) * CH],
                bmask_t[:, mb * CH:(mb + 1) * CH], t[:])
            mloc += 1
            continue
        func = _TILE_FUNC[m]
        end = mloc + 1
        if bias is None:
            while end < 4 and _TILE_FUNC[blk * 4 + end] == func:
                end += 1
        nc.scalar.activation(
            a_new[:, (blk * 4 + mloc) * CH:(blk * 4 + end) * CH],
            ps[:, mloc * CH:end * CH], func, bias=bias_ap)
        mloc = end


def _build_nc():
    nc = bacc.Bacc("TRN2", target_bir_lowering=False, debug=False,
                   num_devices=N_CORES, dynamic_dma_scratch_size=2048)

    # all operands host-packed so each loads as one large contiguous DMA:
    # [p, k*cols + c] = W[k*128 + p, c] for k-tile k
    xT = nc.dram_tensor("xT", [128, KI * BL], BF16, kind="ExternalInput").ap()
    wih = nc.dram_tensor("wih", [128, KI * NH], BF16, kind="ExternalInput").ap()
    whh = nc.dram_tensor("whh", [4 * 128, 4 * NH], BF16,
                         kind="ExternalInput").ap()
    who = nc.dram_tensor("who", [128, KO * KO * NO], BF16,
                         kind="ExternalInput").ap()
    wio = nc.dram_tensor("wio", [128, KI * NO], BF16, kind="ExternalInput").ap()
    hbc = nc.dram_tensor("hbc", [128, KH], F32, kind="ExternalInput").ap()
    obc = nc.dram_tensor("obc", [128, KO], F32, kind="ExternalInput").ap()
    bmask = nc.dram_tensor("bmask", [128, 2 * CH], mybir.dt.uint8,
                           kind="ExternalInput").ap()
    outT = nc.dram_tensor("outT", [NO, BL], F32, kind="ExternalOutput").ap()

    with tile.TileContext(nc) as tc:
        with tc.tile_pool(name="w", bufs=1) as wpool, \
             tc.tile_pool(name="act", bufs=1) as apool, \
             tc.tile_pool(name="ps", bufs=2, space="PSUM") as pspool, \
             tc.tile_pool(name="out", bufs=4) as opool:

            # ---- stage inputs: wih/x split per k-tile for fine-grained
            # deps (compute starts as slices land); whh as 4 big DMAs over
            # both HWDGE queues (SP=sync, ACT=scalar) ----
            wih_m = wpool.tile([128, KI * NH], BF16, tag="projA", name="wihm")
            x_m = wpool.tile([128, KI * BL], BF16, tag="x", name="xm")
            for q in range(4):      # first k-slice lands piecewise
                nc.sync.dma_start(wih_m[:, q * NH // 4:(q + 1) * NH // 4],
                                  wih[:, q * NH // 4:(q + 1) * NH // 4])
            nc.scalar.dma_start(x_m[:, 0:CH], xT[:, 0:CH])
            nc.scalar.dma_start(x_m[:, CH:BL], xT[:, CH:BL])
            for k in range(1, KI):
                for h in range(2):
                    lo = k * NH + h * NH // 2
                    nc.sync.dma_start(wih_m[:, lo:lo + NH // 2],
                                      wih[:, lo:lo + NH // 2])
                    xlo = k * BL + h * CH
                    nc.scalar.dma_start(x_m[:, xlo:xlo + CH],
                                        xT[:, xlo:xlo + CH])
            hbc_t = wpool.tile([128, KH], F32, tag="hbc")
            nc.scalar.dma_start(hbc_t[:], hbc[:])
            obc_t = wpool.tile([128, KO], F32, tag="obc")
            nc.scalar.dma_start(obc_t[:], obc[:])
            bmask_t = wpool.tile([128, 2 * CH], mybir.dt.uint8, tag="bmask")
            nc.scalar.dma_start(bmask_t[:], bmask[:])
            wio_m = wpool.tile([128, KI * NO], BF16, tag="wio", name="wiom")
            nc.scalar.dma_start(wio_m[:], wio[:])
            whh_m = []
            for J in range(4):
                t = wpool.tile([128, 4 * NH], BF16, tag=f"whhJ{J}",
                               name=f"whhJ{J}")
                eng = nc.sync if J % 2 == 0 else nc.scalar
                eng.dma_start(t[:], whh[J * 128:(J + 1) * 128, :])
                whh_m.append(t)
            xT_t = [x_m[:, k * BL:(k + 1) * BL] for k in range(KI)]
            wih_t = [wih_m[:, k * NH:(k + 1) * NH] for k in range(KI)]
            wio_t = [wio_m[:, k * NO:(k + 1) * NO] for k in range(KI)]
            whh_t = [whh_m[k // 4][:, (k % 4) * NH:(k % 4 + 1) * NH]
                     for k in range(KH)]

            # ---- per-chunk x-projection P and first-step activations ----
            P = {}
            A = {}
            for c in range(NCH):
                P[c] = apool.tile([128, KH * CH], BF16, tag=f"P{c}",
                                  name=f"P{c}")
                a1 = apool.tile([128, KH * CH], BF16, tag="A", bufs=3,
                                name=f"A1c{c}")
                for blk in range(4):
                    ps = pspool.tile([128, 4 * CH], F32, tag="ps", name="psb")
                    for k in range(KI):
                        for mloc in range(4):
                            m = blk * 4 + mloc
                            nc.tensor.matmul(
                                ps[:, mloc * CH:(mloc + 1) * CH],
                                wih_t[k][:, m * 128:(m + 1) * 128],
                                xT_t[k][:, c * CH:(c + 1) * CH],
                                start=(k == 0), stop=(k == KI - 1))
                    for mloc in range(4):
                        m = blk * 4 + mloc
                        nc.vector.tensor_scalar_add(
                            P[c][:, m * CH:(m + 1) * CH],
                            ps[:, mloc * CH:(mloc + 1) * CH],
                            hbc_t[:, m:m + 1])
                    # A1 = act(P) straight from SBUF — frees the PSUM slot
                    # as soon as the adds have read it
                    _emit_hidden_act(nc, P[c][:, blk * 4 * CH:(blk + 1) * 4 * CH],
                                     blk, a1, opool, bmask_t)
                A[c] = a1

            # ---- whh-independent output x-projection (fills the window
            # while the 8MB whh load is still in flight) ----
            outx = {}
            for c in range(NCH):
                outx[c] = apool.tile([128, KO * CH], BF16, tag=f"outx{c}",
                                     name=f"outx{c}")
                ps = pspool.tile([128, 4 * CH], F32, tag="ps", name="psb")
                for k in range(KI):
                    for mo in range(KO):
                        nc.tensor.matmul(
                            ps[:, mo * CH:(mo + 1) * CH],
                            wio_t[k][:, mo * 128:(mo + 1) * 128],
                            xT_t[k][:, c * CH:(c + 1) * CH],
                            start=(k == 0), stop=(k == KI - 1))
                nc.vector.tensor_copy(outx[c][:], ps[:])

            # ---- recurrent steps 2..4 ----
            def hh_step(c, s):
                a_new = apool.tile([128, KH * CH], BF16, tag="A", bufs=3,
                                   name=f"A{s + 2}c{c}")
                for blk in range(4):
                    ps = pspool.tile([128, 4 * CH], F32, tag="ps", name="psb")
                    for k in range(KH):
                        for mloc in range(4):
                            m = blk * 4 + mloc
                            nc.tensor.matmul(
                                ps[:, mloc * CH:(mloc + 1) * CH],
                                whh_t[k][:, m * 128:(m + 1) * 128],
                                A[c][:, k * CH:(k + 1) * CH],
                                start=(k == 0), stop=(k == KH - 1))
                    # pre = psum + P into an SBUF temp: a single PSUM read
                    # frees the bank; ACT then runs off SBUF
                    tmp = opool.tile([128, 4 * CH], F32, tag="pre", bufs=2,
                                     name="pre")
                    nc.vector.tensor_add(
                        tmp[:], ps[:], P[c][:, blk * 4 * CH:(blk + 1) * 4 * CH])
                    _emit_hidden_act(nc, tmp, blk, a_new, opool, bmask_t)
                A[c] = a_new

            for s in range(N_STEPS - 2):
                for c in range(NCH):
                    hh_step(c, s)
            hh_step(0, N_STEPS - 2)  # chunk 1's final step emitted after who

            # ---- output layer (who reuses the wih slot); chunk 0's
            # output overlaps chunk 1's final hh step ----
            who_m = wpool.tile([128, KO * KO * NO], BF16, tag="projA",
                               name="whom")
            nc.scalar.dma_start(who_m[:], who[:])
            who_t = [who_m[:, j * KO * NO:(j + 1) * KO * NO]
                     for j in range(KO)]

            def out_chunk(c):
                for mo in range(KO):
                    pso = pspool.tile([128, CH], F32, tag="ps", name="pso")
                    oap = pso[:]
                    for kk in range(KH):
                        j, sj = divmod(kk, KO)
                        nc.tensor.matmul(
                            oap,
                            who_t[j][:, sj * NO + mo * 128:
                                     sj * NO + (mo + 1) * 128],
                            A[c][:, kk * CH:(kk + 1) * CH],
                            start=(kk == 0), stop=(kk == KH - 1))
                    to = opool.tile([128, CH], F32, tag="preo", bufs=2,
                                    name="preo")
                    nc.vector.tensor_add(
                        to[:], oap, outx[c][:, mo * CH:(mo + 1) * CH])
                    o = opool.tile([128, CH], F32, tag="o", bufs=2, name="o")
                    nc.scalar.activation(o[:], to[:], AF.Sigmoid,
                                         bias=obc_t[:, mo:mo + 1])
                    eng = nc.sync if mo % 2 == 0 else nc.scalar
                    eng.dma_start(
                        outT[mo * 128:(mo + 1) * 128, c * CH:(c + 1) * CH],
                        o[:])

            hh_step(1, N_STEPS - 2)
            out_chunk(0)
            out_chunk(1)

    nc.compile()
    return nc


_NC_CACHE = None


def _get_nc():
    global _NC_CACHE
    if _NC_CACHE is None:
        _NC_CACHE = _build_nc()
    return _NC_CACHE


def _make_bmask():
    m = np.zeros((128, 2 * CH), np.uint8)
    m[:_B1 - (_B1 // 128) * 128, 0:CH] = 1          # tile 5: parts < 43 tanh
    m[_B2 - (_B2 // 128) * 128:, CH:2 * CH] = 1     # tile 10: parts >= 86 relu
    return m


def _prep_in_maps(inputs):
    bf = ml_dtypes.bfloat16
    x = np.asarray(inputs["inputs"], np.float32)
    hr = np.asarray(inputs["hidden_responses"], np.float32)[PERM]
    hb = np.asarray(inputs["hidden_biases"], np.float32)[PERM]
    orr = np.asarray(inputs["output_responses"], np.float32)
    ob = np.asarray(inputs["output_biases"], np.float32)

    wih_s = (hr[:, None] * np.asarray(inputs["input_to_hidden"], np.float32)[PERM]).T
    whh_s = (hr[:, None] *
             np.asarray(inputs["hidden_to_hidden"], np.float32)[PERM][:, PERM]).T
    who_s = (orr[:, None] *
             np.asarray(inputs["hidden_to_output"], np.float32)[:, PERM]).T
    wio_s = (orr[:, None] * np.asarray(inputs["input_to_output"], np.float32)).T

    def pack(w, ktiles):     # (ktiles*128, C) -> (128, ktiles*C)
        c = w.shape[1]
        return np.ascontiguousarray(
            w.reshape(ktiles, 128, c).transpose(1, 0, 2).reshape(128, ktiles * c))

    # who: SBUF group j holds k-tiles 4j..4j+3 side by side
    who_p = who_s.reshape(KO, KO, 128, NO).transpose(0, 2, 1, 3).reshape(NO, KO * NO)
    # whh: row-block J packs k-tiles 4J..4J+3
    whh_p = whh_s.reshape(4, 4, 128, NH).transpose(0, 2, 1, 3).reshape(4 * 128, 4 * NH)

    shared = {
        "wih": pack(wih_s, KI).astype(bf),
        "whh": np.ascontiguousarray(whh_p).astype(bf),
        "who": pack(np.ascontiguousarray(who_p), KO).astype(bf),
        "wio": pack(wio_s, KI).astype(bf),
        "hbc": np.ascontiguousarray(hb.reshape(KH, 128).T),
        "obc": np.ascontiguousarray(ob.reshape(KO, 128).T),
        "bmask": _make_bmask(),
    }
    in_maps = []
    for c in range(N_CORES):
        m = dict(shared)
        m["xT"] = pack(np.ascontiguousarray(x[c * BL:(c + 1) * BL].T),
                       KI).astype(bf)
        in_maps.append(m)
    return in_maps


def _run(inputs, trace=False, tmpdir=None):
    nc = _get_nc()
    in_maps = _prep_in_maps(inputs)
    res = run_bass_kernel_spmd(nc, in_maps, core_ids=list(range(N_CORES)),
                               trace=trace, tmpdir=tmpdir)
    out = np.empty((B, NO), np.float32)
    for c in range(N_CORES):
        out[c * BL:(c + 1) * BL] = res.results[c]["outT"].T
    return out, res


def kernel(**inputs) -> np.ndarray:
    out, _ = _run(inputs, trace=False)
    return out


if __name__ == "__main__":
    rng = np.random.default_rng(0)
    ins = {
        "inputs": rng.standard_normal((B, NI), dtype=np.float32),
        "input_to_hidden": rng.standard_normal((NH, NI), dtype=np.float32) * 0.02,
        "hidden_to_hidden": rng.standard_normal((NH, NH), dtype=np.float32) * 0.02,
        "output_to_hidden": rng.standard_normal((NH, NO), dtype=np.float32) * 0.02,
        "input_to_output": rng.standard_normal((NO, NI), dtype=np.float32) * 0.02,
        "hidden_to_output": rng.standard_normal((NO, NH), dtype=np.float32) * 0.02,
        "output_to_output": rng.standard_normal((NO, NO), dtype=np.float32) * 0.02,
        "hidden_responses": rng.standard_normal(NH, dtype=np.float32) * 0.1 + 1.0,
        "hidden_biases": rng.standard_normal(NH, dtype=np.float32) * 0.1,
        "output_responses": rng.standard_normal(NO, dtype=np.float32) * 0.1 + 1.0,
        "output_biases": rng.standard_normal(NO, dtype=np.float32) * 0.1,
    }
    out = kernel(**ins)
    print("kernel output", out.shape, out.dtype, out[:2, :4])



# revision 2
# speedup vs baseline: 1.6940x; 1.6940x over previous
"""Trainium2 Bass kernel for a 4-step differentiable recurrent net forward pass.

Reference computation (B=8192, NI=512, NH=2048, NO=512, 4 steps):
    activs = 0; outputs = 0
    repeat 4x:  pre = hr * (x @ Wih.T + activs @ Whh.T + outputs @ Woh.T) + hb
                activs = per_neuron_act(pre)        # tanh/sigmoid/relu by i%3
    out = sigmoid(or * (x @ Wio.T + outputs @ Woo.T + activs @ Who.T) + ob)

`outputs` is never written inside the loop, so the Woh/Woo terms vanish and
the x-projection P = hr*(x@Wih.T)+hb is loop-invariant (computed once).

Strategy: data-parallel on batch across 8 cores (1024 rows each). On-core
everything is feature-major (features on SBUF partitions, batch on the free
axis), so each matmul is W_tile.T @ X^T with stationary weights. The
recurrent Whh and the output Who matmuls (91% of tensor work) run in
fp8 e4m3 with perf_mode=DoubleRow: K=256 contraction per instruction at 2
MACs/PE-cycle. Weights are scaled x32 before the fp8 cast (keeps them out
of the subnormal range); every PSUM eviction goes through the ACT engine
with scale=1/32 to compensate. Activations are written as fp8 directly by
the ACT engine. The input projections (x@Wih, x@Wio) stay bf16 for
accuracy. Host-side prep: hidden neurons are permuted so the three
activation groups are contiguous, hr/or are folded into the weight
matrices, weights are packed so each loads as one large contiguous DMA,
and hb/ob are applied as per-partition bias APs.
"""

import os

import numpy as np
import ml_dtypes

import concourse.bass as bass
import concourse.tile as tile
from concourse import bacc, mybir
from concourse.bass_utils import run_bass_kernel_spmd

B, NI, NH, NO = 8192, 512, 2048, 512
N_STEPS = 4
N_CORES = 8
BL = B // N_CORES          # batch rows per core
CH = 512                   # batch chunk (one PSUM bank of fp32)
NCH = BL // CH             # 2 chunks per core
KI = NI // 128             # 4 k-tiles over inputs
KH = NH // 128             # 16 k/m-tiles over hidden
KP = KH // 2               # 8 DoubleRow k-pairs over hidden
KO = NO // 128             # 4 m-tiles over outputs

BF16 = mybir.dt.bfloat16
F32 = mybir.dt.float32
FP8 = mybir.dt.float8e4
AF = mybir.ActivationFunctionType
DR = mybir.MatmulPerfMode.DoubleRow

SCALE = 32.0               # fp8 weight pre-scale; undone by ACT scale=1/SCALE
INV = 1.0 / SCALE

# hidden neurons regrouped as [all tanh | all sigmoid | all relu]
_idx = np.arange(NH)
PERM = np.concatenate([_idx[_idx % 3 == 0], _idx[_idx % 3 == 1], _idx[_idx % 3 == 2]])
_B1 = int((_idx % 3 == 0).sum())           # 683
_B2 = _B1 + int((_idx % 3 == 1).sum())     # 1366

# per m-tile: the single activation function, or None for the two mixed tiles
_TILE_FUNC = []
for _m in range(KH):
    _lo, _hi = _m * 128, (_m + 1) * 128
    _fs = set()
    for _f, _a, _b in ((AF.Tanh, 0, _B1), (AF.Sigmoid, _B1, _B2), (AF.Relu, _B2, NH)):
        if max(_lo, _a) < min(_hi, _b):
            _fs.add(_f)
    _TILE_FUNC.append(_fs.pop() if len(_fs) == 1 else None)

# mixed tiles: (major_func applied everywhere, minor_func, mask column block)
# partition sub-ranges must be 32-aligned on TRN2, so the minority strip is
# fixed up with a full-tile ACT + copy_predicated against a {0,1} mask
_BOUNDARY = {
    _B1 // 128: (AF.Sigmoid, AF.Tanh, 0),    # tile 5: parts < 43 are tanh
    _B2 // 128: (AF.Sigmoid, AF.Relu, 1),    # tile 10: parts >= 86 are relu
}


def _emit_hidden_act(nc, ps, blk, a_new, tmp_pool, bmask_t):
    """Evict a 4-m-tile pre-activation block through the grouped activations.

    ps:    SBUF/PSUM AP (128, 4*CH) f32 holding SCALE*pre for m-tiles
           blk*4..blk*4+3
    a_new: SBUF tile (128, KH, CH) fp8, m-tile m lives at [:, m, :]
    """
    mloc = 0
    while mloc < 4:
        m = blk * 4 + mloc
        if m in _BOUNDARY:
            major, minor, mb = _BOUNDARY[m]
            nc.scalar.activation(
                a_new[:, m, :], ps[:, mloc * CH:(mloc + 1) * CH], major,
                scale=INV)
            t = tmp_pool.tile([128, CH], FP8, tag="btmp", bufs=2, name="btmp")
            nc.scalar.activation(t[:], ps[:, mloc * CH:(mloc + 1) * CH], minor,
                                 scale=INV)
            nc.vector.copy_predicated(
                a_new[:, m, :], bmask_t[:, mb * CH:(mb + 1) * CH], t[:])
            mloc += 1
            continue
        func = _TILE_FUNC[m]
        end = mloc + 1
        while end < 4 and _TILE_FUNC[blk * 4 + end] == func:
            end += 1
        nc.scalar.activation(
            a_new[:, blk * 4 + mloc:blk * 4 + end, :],
            ps[:, mloc * CH:end * CH], func, scale=INV)
        mloc = end


def _build_nc():
    nc = bacc.Bacc("TRN2", target_bir_lowering=False, debug=False,
                   num_devices=N_CORES, dynamic_dma_scratch_size=2048)

    # all operands host-packed so each loads as one large contiguous DMA:
    # wih/x/wio: [p, k*cols + c] = W[k*128 + p, c] for k-tile k
    # whh (fp8): [p, (t*16+m)*2+i, j] = Whh_s[(2t+i)*128+p, m*128+j]
    # who (fp8): [p, (t*4+mo)*2+i, j] = Who_s[(2t+i)*128+p, mo*128+j]
    xT = nc.dram_tensor("xT", [128, KI * BL], BF16, kind="ExternalInput").ap()
    wih = nc.dram_tensor("wih", [128, KI * NH], BF16, kind="ExternalInput").ap()
    whh = nc.dram_tensor("whh", [128, KP * KH * 2, 128], FP8,
                         kind="ExternalInput").ap()
    who = nc.dram_tensor("who", [128, KP * KO * 2, 128], FP8,
                         kind="ExternalInput").ap()
    wio = nc.dram_tensor("wio", [128, KI * NO], BF16, kind="ExternalInput").ap()
    hbc = nc.dram_tensor("hbc", [128, KH], F32, kind="ExternalInput").ap()
    obc = nc.dram_tensor("obc", [128, KO], F32, kind="ExternalInput").ap()
    bmask = nc.dram_tensor("bmask", [128, 2 * CH], mybir.dt.uint8,
                           kind="ExternalInput").ap()
    outT = nc.dram_tensor("outT", [NO, BL], F32, kind="ExternalOutput").ap()

    with tile.TileContext(nc) as tc:
        with tc.tile_pool(name="w", bufs=1) as wpool, \
             tc.tile_pool(name="act", bufs=1) as apool, \
             tc.tile_pool(name="ps", bufs=2, space="PSUM") as pspool, \
             tc.tile_pool(name="out", bufs=4) as opool:

            # ---- stage inputs: wih/x split per k-tile for fine-grained
            # deps (compute starts as slices land); whh as 4 big DMAs over
            # both HWDGE queues (SP=sync, ACT=scalar) ----
            wih_m = wpool.tile([128, KI * NH], BF16, tag="wih", name="wihm")
            x_m = wpool.tile([128, KI * BL], BF16, tag="x", name="xm")
            for q in range(4):      # first k-slice lands piecewise
                nc.sync.dma_start(wih_m[:, q * NH // 4:(q + 1) * NH // 4],
                                  wih[:, q * NH // 4:(q + 1) * NH // 4])
            nc.scalar.dma_start(x_m[:, 0:CH], xT[:, 0:CH])
            nc.scalar.dma_start(x_m[:, CH:BL], xT[:, CH:BL])
            for k in range(1, KI):
                for h in range(2):
                    lo = k * NH + h * NH // 2
                    nc.sync.dma_start(wih_m[:, lo:lo + NH // 2],
                                      wih[:, lo:lo + NH // 2])
                    xlo = k * BL + h * CH
                    nc.scalar.dma_start(x_m[:, xlo:xlo + CH],
                                        xT[:, xlo:xlo + CH])
            hbc_t = wpool.tile([128, KH], F32, tag="hbc")
            nc.scalar.dma_start(hbc_t[:], hbc[:])
            obc_t = wpool.tile([128, KO], F32, tag="obc")
            nc.scalar.dma_start(obc_t[:], obc[:])
            bmask_t = wpool.tile([128, 2 * CH], mybir.dt.uint8, tag="bmask")
            nc.scalar.dma_start(bmask_t[:], bmask[:])
            wio_m = wpool.tile([128, KI * NO], BF16, tag="wio", name="wiom")
            nc.scalar.dma_start(wio_m[:], wio[:])
            # whh fp8: 4MB total as 4x 1MB DMAs alternating queues
            whh_m = wpool.tile([128, KP * KH * 2, 128], FP8, tag="whh",
                               name="whhm")
            for J in range(4):
                eng = nc.sync if J % 2 == 0 else nc.scalar
                eng.dma_start(whh_m[:, J * 64:(J + 1) * 64, :],
                              whh[:, J * 64:(J + 1) * 64, :])
            who_m = wpool.tile([128, KP * KO * 2, 128], FP8, tag="who",
                               name="whom")
            nc.scalar.dma_start(who_m[:], who[:])
            xT_t = [x_m[:, k * BL:(k + 1) * BL] for k in range(KI)]
            wih_t = [wih_m[:, k * NH:(k + 1) * NH] for k in range(KI)]
            wio_t = [wio_m[:, k * NO:(k + 1) * NO] for k in range(KI)]

            # ---- per-chunk x-projection P (= SCALE*(hr*(x@Wih.T)+hb), bf16)
            # and first-step activations ----
            P = {}
            A = {}
            for c in range(NCH):
                P[c] = apool.tile([128, KH * CH], BF16, tag=f"P{c}",
                                  name=f"P{c}")
                a1 = apool.tile([128, KH, CH], FP8, tag="A", bufs=3,
                                name=f"A1c{c}")
                for blk in range(4):
                    ps = pspool.tile([128, 4 * CH], F32, tag="ps", name="psb")
                    for k in range(KI):
                        for mloc in range(4):
                            m = blk * 4 + mloc
                            nc.tensor.matmul(
                                ps[:, mloc * CH:(mloc + 1) * CH],
                                wih_t[k][:, m * 128:(m + 1) * 128],
                                xT_t[k][:, c * CH:(c + 1) * CH],
                                start=(k == 0), stop=(k == KI - 1))
                    for mloc in range(4):
                        m = blk * 4 + mloc
                        nc.vector.tensor_scalar_add(
                            P[c][:, m * CH:(m + 1) * CH],
                            ps[:, mloc * CH:(mloc + 1) * CH],
                            hbc_t[:, m:m + 1])
                    # A1 = act(P/SCALE) straight from SBUF — frees the PSUM
                    # slot as soon as the adds have read it
                    _emit_hidden_act(nc, P[c][:, blk * 4 * CH:(blk + 1) * 4 * CH],
                                     blk, a1, opool, bmask_t)
                A[c] = a1

            # ---- whh-independent output x-projection (fills the window
            # while the 4MB whh load is still in flight); holds SCALE*or*
            # (x@Wio.T) ----
            outx = {}
            for c in range(NCH):
                outx[c] = apool.tile([128, KO * CH], BF16, tag=f"outx{c}",
                                     name=f"outx{c}")
                ps = pspool.tile([128, 4 * CH], F32, tag="ps", name="psb")
                for k in range(KI):
                    for mo in range(KO):
                        nc.tensor.matmul(
                            ps[:, mo * CH:(mo + 1) * CH],
                            wio_t[k][:, mo * 128:(mo + 1) * 128],
                            xT_t[k][:, c * CH:(c + 1) * CH],
                            start=(k == 0), stop=(k == KI - 1))
                nc.vector.tensor_copy(outx[c][:], ps[:])

            # ---- recurrent steps 2..4: fp8 DoubleRow, K=256/instruction ----
            def hh_step(c, s):
                a_new = apool.tile([128, KH, CH], FP8, tag="A", bufs=3,
                                   name=f"A{s + 2}c{c}")
                for blk in range(4):
                    ps = pspool.tile([128, 4 * CH], F32, tag="ps", name="psb")
                    for t in range(KP):
                        for mloc in range(4):
                            m = blk * 4 + mloc
                            w2 = (t * KH + m) * 2
                            nc.tensor.matmul(
                                ps[:, mloc * CH:(mloc + 1) * CH],
                                whh_m[:, w2:w2 + 2, :],
                                A[c][:, 2 * t:2 * t + 2, :],
                                start=(t == 0), stop=(t == KP - 1),
                                perf_mode=DR)
                    # pre = psum + P into an SBUF temp: a single PSUM read
                    # frees the bank; ACT then runs off SBUF
                    tmp = opool.tile([128, 4 * CH], F32, tag="pre", bufs=2,
                                     name="pre")
                    nc.vector.tensor_add(
                        tmp[:], ps[:], P[c][:, blk * 4 * CH:(blk + 1) * 4 * CH])
                    _emit_hidden_act(nc, tmp, blk, a_new, opool, bmask_t)
                A[c] = a_new

            for s in range(N_STEPS - 2):
                for c in range(NCH):
                    hh_step(c, s)
            hh_step(0, N_STEPS - 2)  # chunk 1's final step emitted after who

            # ---- output layer (fp8 DoubleRow); chunk 0's output overlaps
            # chunk 1's final hh step ----
            def out_chunk(c):
                for mo in range(KO):
                    pso = pspool.tile([128, CH], F32, tag="ps", name="pso")
                    for t in range(KP):
                        w2 = (t * KO + mo) * 2
                        nc.tensor.matmul(
                            pso[:],
                            who_m[:, w2:w2 + 2, :],
                            A[c][:, 2 * t:2 * t + 2, :],
                            start=(t == 0), stop=(t == KP - 1),
                            perf_mode=DR)
                    to = opool.tile([128, CH], F32, tag="preo", bufs=2,
                                    name="preo")
                    nc.vector.tensor_add(
                        to[:], pso[:], outx[c][:, mo * CH:(mo + 1) * CH])
                    o = opool.tile([128, CH], F32, tag="o", bufs=2, name="o")
                    nc.scalar.activation(o[:], to[:], AF.Sigmoid,
                                         bias=obc_t[:, mo:mo + 1], scale=INV)
                    eng = nc.sync if mo % 2 == 0 else nc.scalar
                    eng.dma_start(
                        outT[mo * 128:(mo + 1) * 128, c * CH:(c + 1) * CH],
                        o[:])

            hh_step(1, N_STEPS - 2)
            out_chunk(0)
            out_chunk(1)

    nc.compile()
    return nc


_NC_CACHE = None


def _get_nc():
    global _NC_CACHE
    if _NC_CACHE is None:
        _NC_CACHE = _build_nc()
    return _NC_CACHE


def _make_bmask():
    m = np.zeros((128, 2 * CH), np.uint8)
    m[:_B1 - (_B1 // 128) * 128, 0:CH] = 1          # tile 5: parts < 43 tanh
    m[_B2 - (_B2 // 128) * 128:, CH:2 * CH] = 1     # tile 10: parts >= 86 relu
    return m


def _pack_dr(w_s, mt):
    """(NH, mt*128) k-major weights -> DoubleRow layout [128, KP*mt*2, 128].

    [p, (t*mt+m)*2+i, j] = w_s[(2t+i)*128 + p, m*128 + j]
    """
    fp8 = ml_dtypes.float8_e4m3
    w4 = w_s.reshape(KP, 2, 128, mt, 128)            # t, i, p, m, j
    return np.ascontiguousarray(
        w4.transpose(2, 0, 3, 1, 4).reshape(128, KP * mt * 2, 128)).astype(fp8)


def _prep_in_maps(inputs):
    bf = ml_dtypes.bfloat16
    x = np.asarray(inputs["inputs"], np.float32)
    hr = np.asarray(inputs["hidden_responses"], np.float32)[PERM]
    hb = np.asarray(inputs["hidden_biases"], np.float32)[PERM]
    orr = np.asarray(inputs["output_responses"], np.float32)
    ob = np.asarray(inputs["output_biases"], np.float32)

    wih_s = SCALE * (hr[:, None] *
                     np.asarray(inputs["input_to_hidden"], np.float32)[PERM]).T
    whh_s = SCALE * (hr[:, None] *
                     np.asarray(inputs["hidden_to_hidden"],
                                np.float32)[PERM][:, PERM]).T
    who_s = SCALE * (orr[:, None] *
                     np.asarray(inputs["hidden_to_output"],
                                np.float32)[:, PERM]).T
    wio_s = SCALE * (orr[:, None] *
                     np.asarray(inputs["input_to_output"], np.float32)).T

    def pack(w, ktiles):     # (ktiles*128, C) -> (128, ktiles*C)
        c = w.shape[1]
        return np.ascontiguousarray(
            w.reshape(ktiles, 128, c).transpose(1, 0, 2).reshape(128, ktiles * c))

    shared = {
        "wih": pack(wih_s, KI).astype(bf),
        "whh": _pack_dr(whh_s, KH),
        "who": _pack_dr(who_s, KO),
        "wio": pack(wio_s, KI).astype(bf),
        "hbc": np.ascontiguousarray(SCALE * hb.reshape(KH, 128).T),
        "obc": np.ascontiguousarray(ob.reshape(KO, 128).T),
        "bmask": _make_bmask(),
    }
    in_maps = []
    for c in range(N_CORES):
        m = dict(shared)
        m["xT"] = pack(np.ascontiguousarray(x[c * BL:(c + 1) * BL].T),
                       KI).astype(bf)
        in_maps.append(m)
    return in_maps


def _run(inputs, trace=False, tmpdir=None):
    nc = _get_nc()
    in_maps = _prep_in_maps(inputs)
    res = run_bass_kernel_spmd(nc, in_maps, core_ids=list(range(N_CORES)),
                               trace=trace, tmpdir=tmpdir)
    out = np.empty((B, NO), np.float32)
    for c in range(N_CORES):
        out[c * BL:(c + 1) * BL] = res.results[c]["outT"].T
    return out, res


def kernel(**inputs) -> np.ndarray:
    out, _ = _run(inputs, trace=False)
    return out


if __name__ == "__main__":
    rng = np.random.default_rng(0)
    ins = {
        "inputs": rng.standard_normal((B, NI), dtype=np.float32),
        "input_to_hidden": rng.standard_normal((NH, NI), dtype=np.float32) * 0.02,
        "hidden_to_hidden": rng.standard_normal((NH, NH), dtype=np.float32) * 0.02,
        "output_to_hidden": rng.standard_normal((NH, NO), dtype=np.float32) * 0.02,
        "input_to_output": rng.standard_normal((NO, NI), dtype=np.float32) * 0.02,
        "hidden_to_output": rng.standard_normal((NO, NH), dtype=np.float32) * 0.02,
        "output_to_output": rng.standard_normal((NO, NO), dtype=np.float32) * 0.02,
        "hidden_responses": rng.standard_normal(NH, dtype=np.float32) * 0.1 + 1.0,
        "hidden_biases": rng.standard_normal(NH, dtype=np.float32) * 0.1,
        "output_responses": rng.standard_normal(NO, dtype=np.float32) * 0.1 + 1.0,
        "output_biases": rng.standard_normal(NO, dtype=np.float32) * 0.1,
    }
    out = kernel(**ins)
    print("kernel output", out.shape, out.dtype, out[:2, :4])


# revision 3
# speedup vs baseline: 1.7588x; 1.0382x over previous
"""Trainium2 Bass kernel for a 4-step differentiable recurrent net forward pass.

Reference computation (B=8192, NI=512, NH=2048, NO=512, 4 steps):
    activs = 0; outputs = 0
    repeat 4x:  pre = hr * (x @ Wih.T + activs @ Whh.T + outputs @ Woh.T) + hb
                activs = per_neuron_act(pre)        # tanh/sigmoid/relu by i%3
    out = sigmoid(or * (x @ Wio.T + outputs @ Woo.T + activs @ Who.T) + ob)

`outputs` is never written inside the loop, so the Woh/Woo terms vanish and
the x-projection P = hr*(x@Wih.T)+hb is loop-invariant (computed once).

Strategy: data-parallel on batch across 8 cores (1024 rows each). On-core
everything is feature-major (features on SBUF partitions, batch on the free
axis), so each matmul is W_tile.T @ X^T with stationary weights. The
recurrent Whh and the output Who matmuls (91% of tensor work) run in
fp8 e4m3 with perf_mode=DoubleRow: K=256 contraction per instruction at 2
MACs/PE-cycle. DoubleRow LDWEIGHTS (256 columns, no FWL) costs more than
the matmul itself, so both 512-row batch chunks are computed back-to-back
under one weight load: bass emits an InstLdweights per matmul, and a
post-build pass (_dedupe_ldweights) removes the redundant second load so
the non-self-loading second matmul reuses the array-resident weights.
Weights are scaled x32 before the fp8 cast (keeps them out of the
subnormal range); every PSUM eviction goes through the ACT engine with
scale=1/32 to compensate. Eviction temporaries are fp16 (the ACT engine
reads 2B/cycle/lane, so f32 sources run at half rate) and activations are
written as fp8 directly by the ACT engine. The input projections (x@Wih,
x@Wio) stay bf16 for accuracy. Host-side prep: hidden neurons are permuted
so the three activation groups are contiguous, hr/or are folded into the
weight matrices, weights are packed so each loads as one large contiguous
DMA, and hb/ob are applied as per-partition bias APs.
"""

import os

import numpy as np
import ml_dtypes

import concourse.bass as bass
import concourse.tile as tile
from concourse import bacc, mybir
from concourse.bass_utils import run_bass_kernel_spmd

B, NI, NH, NO = 8192, 512, 2048, 512
N_STEPS = 4
N_CORES = 8
BL = B // N_CORES          # batch rows per core
CH = 512                   # batch chunk (max moving free dim)
NCH = BL // CH             # 2 chunks per core
KI = NI // 128             # 4 k-tiles over inputs
KH = NH // 128             # 16 k/m-tiles over hidden
KP = KH // 2               # 8 DoubleRow k-pairs over hidden
KO = NO // 128             # 4 m-tiles over outputs

BF16 = mybir.dt.bfloat16
F16 = mybir.dt.float16
F32 = mybir.dt.float32
FP8 = mybir.dt.float8e4
AF = mybir.ActivationFunctionType
DR = mybir.MatmulPerfMode.DoubleRow

SCALE = 32.0               # fp8 weight pre-scale; undone by ACT scale=1/SCALE
INV = 1.0 / SCALE

# hidden neurons regrouped as [all tanh | all sigmoid | all relu]
_idx = np.arange(NH)
PERM = np.concatenate([_idx[_idx % 3 == 0], _idx[_idx % 3 == 1], _idx[_idx % 3 == 2]])
_B1 = int((_idx % 3 == 0).sum())           # 683
_B2 = _B1 + int((_idx % 3 == 1).sum())     # 1366

# per m-tile: the single activation function, or None for the two mixed tiles
_TILE_FUNC = []
for _m in range(KH):
    _lo, _hi = _m * 128, (_m + 1) * 128
    _fs = set()
    for _f, _a, _b in ((AF.Tanh, 0, _B1), (AF.Sigmoid, _B1, _B2), (AF.Relu, _B2, NH)):
        if max(_lo, _a) < min(_hi, _b):
            _fs.add(_f)
    _TILE_FUNC.append(_fs.pop() if len(_fs) == 1 else None)

# mixed tiles: (major_func applied everywhere, minor_func, mask column block)
# partition sub-ranges must be 32-aligned on TRN2, so the minority strip is
# fixed up with a full-tile ACT + copy_predicated against a {0,1} mask
_BOUNDARY = {
    _B1 // 128: (AF.Sigmoid, AF.Tanh, 0),    # tile 5: parts < 43 are tanh
    _B2 // 128: (AF.Sigmoid, AF.Relu, 1),    # tile 10: parts >= 86 are relu
}


def _dedupe_ldweights(nc):
    """Drop an InstLdweights that reloads exactly what the PE already holds.

    bass splits every matmul into InstLdweights + non-self-loading
    InstMatmult at build time; consecutive matmuls on the same stationary
    tile therefore carry a redundant (and expensive, for DoubleRow) reload.
    Only sync-free duplicates are removed, and any non-matmul PE
    instruction invalidates the tracked weights.
    """
    removed = 0
    for blk in nc.main_func.blocks:
        prev_key = None
        to_remove = []
        for i in blk.instructions:
            tn = type(i).__name__
            if tn == "InstLdweights":
                k = (repr(i.ins[0]), repr(i.perf_mode), repr(i.is_transpose),
                     repr(i.tile_position), repr(i.tile_size))
                si = i.sync_info
                clean = si is None or (len(si.on_wait) == 0
                                       and len(si.on_update) == 0)
                if k == prev_key and clean:
                    to_remove.append(i)
                else:
                    prev_key = k
            elif tn == "InstMatmult":
                continue
            elif getattr(i, "engine", None) == mybir.EngineType.PE:
                prev_key = None
        for i in to_remove:
            blk.instructions.remove(i)
        removed += len(to_remove)
    return removed


def _emit_hidden_act2(nc, ps, blk2, a_new, tmp_pool, bmask_t):
    """Evict a 2-m-tile pre-activation slab through the grouped activations.

    ps:    SBUF AP (128, 2*CH) fp16/bf16 holding SCALE*pre for m-tiles
           2*blk2, 2*blk2+1
    a_new: SBUF tile (128, KH, CH) fp8, m-tile m lives at [:, m, :]
    """
    mloc = 0
    while mloc < 2:
        m = 2 * blk2 + mloc
        if m in _BOUNDARY:
            major, minor, mb = _BOUNDARY[m]
            nc.scalar.activation(
                a_new[:, m, :], ps[:, mloc * CH:(mloc + 1) * CH], major,
                scale=INV)
            t = tmp_pool.tile([128, CH], FP8, tag="btmp", bufs=2, name="btmp")
            nc.scalar.activation(t[:], ps[:, mloc * CH:(mloc + 1) * CH], minor,
                                 scale=INV)
            nc.vector.copy_predicated(
                a_new[:, m, :], bmask_t[:, mb * CH:(mb + 1) * CH], t[:])
            mloc += 1
            continue
        func = _TILE_FUNC[m]
        end = mloc + 1
        while end < 2 and _TILE_FUNC[2 * blk2 + end] == func:
            end += 1
        nc.scalar.activation(
            a_new[:, 2 * blk2 + mloc:2 * blk2 + end, :],
            ps[:, mloc * CH:end * CH], func, scale=INV)
        mloc = end


def _build_nc():
    nc = bacc.Bacc("TRN2", target_bir_lowering=False, debug=False,
                   num_devices=N_CORES, dynamic_dma_scratch_size=2048)

    # all operands host-packed so each loads as one large contiguous DMA:
    # wih/x/wio: [p, k*cols + c] = W[k*128 + p, c] for k-tile k
    # whh (fp8): [p, (t*16+m)*2+i, j] = Whh_s[(2t+i)*128+p, m*128+j]
    # who (fp8): [p, (t*4+mo)*2+i, j] = Who_s[(2t+i)*128+p, mo*128+j]
    xT = nc.dram_tensor("xT", [128, KI * BL], BF16, kind="ExternalInput").ap()
    wih = nc.dram_tensor("wih", [128, KI * NH], BF16, kind="ExternalInput").ap()
    whh = nc.dram_tensor("whh", [128, KP * KH * 2, 128], FP8,
                         kind="ExternalInput").ap()
    who = nc.dram_tensor("who", [128, KP * KO * 2, 128], FP8,
                         kind="ExternalInput").ap()
    wio = nc.dram_tensor("wio", [128, KI * NO], BF16, kind="ExternalInput").ap()
    hbc = nc.dram_tensor("hbc", [128, KH], F32, kind="ExternalInput").ap()
    obc = nc.dram_tensor("obc", [128, KO], F32, kind="ExternalInput").ap()
    bmask = nc.dram_tensor("bmask", [128, 2 * CH], mybir.dt.uint8,
                           kind="ExternalInput").ap()
    outT = nc.dram_tensor("outT", [NO, BL], BF16, kind="ExternalOutput").ap()

    with tile.TileContext(nc) as tc:
        with tc.tile_pool(name="w", bufs=1) as wpool, \
             tc.tile_pool(name="act", bufs=1) as apool, \
             tc.tile_pool(name="ps", bufs=2, space="PSUM") as pspool, \
             tc.tile_pool(name="out", bufs=4) as opool:

            # ---- stage inputs: wih/x split per k-tile for fine-grained
            # deps (compute starts as slices land); whh as 4 big DMAs over
            # both HWDGE queues (SP=sync, ACT=scalar) ----
            wih_m = wpool.tile([128, KI * NH], BF16, tag="wih", name="wihm")
            x_m = wpool.tile([128, KI * BL], BF16, tag="x", name="xm")
            for q in range(4):      # first k-slice lands piecewise
                nc.sync.dma_start(wih_m[:, q * NH // 4:(q + 1) * NH // 4],
                                  wih[:, q * NH // 4:(q + 1) * NH // 4])
            nc.scalar.dma_start(x_m[:, 0:CH], xT[:, 0:CH])
            nc.scalar.dma_start(x_m[:, CH:BL], xT[:, CH:BL])
            for k in range(1, KI):
                for h in range(2):
                    lo = k * NH + h * NH // 2
                    nc.sync.dma_start(wih_m[:, lo:lo + NH // 2],
                                      wih[:, lo:lo + NH // 2])
                    xlo = k * BL + h * CH
                    nc.scalar.dma_start(x_m[:, xlo:xlo + CH],
                                        xT[:, xlo:xlo + CH])
            hbc_t = wpool.tile([128, KH], F32, tag="hbc")
            nc.scalar.dma_start(hbc_t[:], hbc[:])
            obc_t = wpool.tile([128, KO], F32, tag="obc")
            nc.scalar.dma_start(obc_t[:], obc[:])
            bmask_t = wpool.tile([128, 2 * CH], mybir.dt.uint8, tag="bmask")
            nc.scalar.dma_start(bmask_t[:], bmask[:])
            wio_m = wpool.tile([128, KI * NO], BF16, tag="wio", name="wiom")
            nc.scalar.dma_start(wio_m[:], wio[:])
            # whh fp8: 4MB total as 4x 1MB DMAs alternating queues
            whh_m = wpool.tile([128, KP * KH * 2, 128], FP8, tag="whh",
                               name="whhm")
            for J in range(4):
                eng = nc.sync if J % 2 == 0 else nc.scalar
                eng.dma_start(whh_m[:, J * 64:(J + 1) * 64, :],
                              whh[:, J * 64:(J + 1) * 64, :])
            who_m = wpool.tile([128, KP * KO * 2, 128], FP8, tag="who",
                               name="whom")
            nc.scalar.dma_start(who_m[:], who[:])
            xT_t = [x_m[:, k * BL:(k + 1) * BL] for k in range(KI)]
            wih_t = [wih_m[:, k * NH:(k + 1) * NH] for k in range(KI)]
            wio_t = [wio_m[:, k * NO:(k + 1) * NO] for k in range(KI)]

            def psum2(i):
                # two 2-bank accumulators live at once (one per chunk, or
                # pipelined across 2-m-tile blocks); bufs=2 each fills PSUM
                return pspool.tile([128, 2 * CH], F32,
                                   tag=("psA" if i % 2 == 0 else "psB"),
                                   bufs=2, name="psb")

            # ---- per-chunk x-projection P (= SCALE*(hr*(x@Wih.T)+hb), bf16)
            # and first-step activations, in 2-m-tile blocks ----
            P = {}
            A = {}
            for c in range(NCH):
                P[c] = apool.tile([128, KH * CH], BF16, tag=f"P{c}",
                                  name=f"P{c}")
                a1 = apool.tile([128, KH, CH], FP8, tag=f"A{c}", bufs=2,
                                name=f"A1c{c}")
                for blk in range(8):
                    ps = psum2(blk)
                    for k in range(KI):
                        for mloc in range(2):
                            m = 2 * blk + mloc
                            nc.tensor.matmul(
                                ps[:, mloc * CH:(mloc + 1) * CH],
                                wih_t[k][:, m * 128:(m + 1) * 128],
                                xT_t[k][:, c * CH:(c + 1) * CH],
                                start=(k == 0), stop=(k == KI - 1))
                    for mloc in range(2):
                        m = 2 * blk + mloc
                        nc.vector.tensor_scalar_add(
                            P[c][:, m * CH:(m + 1) * CH],
                            ps[:, mloc * CH:(mloc + 1) * CH],
                            hbc_t[:, m:m + 1])
                    # A1 = act(P/SCALE) straight from SBUF — frees the PSUM
                    # slot as soon as the adds have read it
                    _emit_hidden_act2(nc, P[c][:, 2 * blk * CH:(2 * blk + 2) * CH],
                                      blk, a1, opool, bmask_t)
                A[c] = a1

            # ---- whh-independent output x-projection (fills the window
            # while the 4MB whh load is still in flight); holds SCALE*or*
            # (x@Wio.T) ----
            outx = {}
            for c in range(NCH):
                outx[c] = apool.tile([128, KO * CH], BF16, tag=f"outx{c}",
                                     name=f"outx{c}")
                for half in range(2):
                    ps = psum2(half)
                    for k in range(KI):
                        for mloc in range(2):
                            mo = 2 * half + mloc
                            nc.tensor.matmul(
                                ps[:, mloc * CH:(mloc + 1) * CH],
                                wio_t[k][:, mo * 128:(mo + 1) * 128],
                                xT_t[k][:, c * CH:(c + 1) * CH],
                                start=(k == 0), stop=(k == KI - 1))
                    nc.vector.tensor_copy(
                        outx[c][:, 2 * half * CH:(2 * half + 2) * CH], ps[:])

            # ---- recurrent steps 2..4: fp8 DoubleRow, both chunks computed
            # back-to-back per weight load (the dedupe pass removes the
            # second LDWEIGHTS) ----
            def hh_step_fused(s):
                a_new = [apool.tile([128, KH, CH], FP8, tag=f"A{c}", bufs=2,
                                    name=f"A{s + 2}c{c}") for c in range(NCH)]
                for blk in range(8):
                    pss = [psum2(0), psum2(1)]
                    for t in range(KP):
                        for mloc in range(2):
                            m = 2 * blk + mloc
                            w2 = (t * KH + m) * 2
                            for c in range(NCH):
                                nc.tensor.matmul(
                                    pss[c][:, mloc * CH:(mloc + 1) * CH],
                                    whh_m[:, w2:w2 + 2, :],
                                    A[c][:, 2 * t:2 * t + 2, :],
                                    start=(t == 0), stop=(t == KP - 1),
                                    perf_mode=DR)
                    for c in range(NCH):
                        # pre = psum + P into an fp16 SBUF temp: a single
                        # PSUM read frees the bank, and the ACT engine reads
                        # 16-bit sources at full rate
                        tmp = opool.tile([128, 2 * CH], F16, tag="pre",
                                         bufs=4, name="pre")
                        nc.vector.tensor_add(
                            tmp[:], pss[c][:],
                            P[c][:, 2 * blk * CH:(2 * blk + 2) * CH])
                        _emit_hidden_act2(nc, tmp, blk, a_new[c], opool,
                                          bmask_t)
                for c in range(NCH):
                    A[c] = a_new[c]

            for s in range(N_STEPS - 1):
                hh_step_fused(s)

            # ---- output layer (fp8 DoubleRow, same weight reuse) ----
            for mo in range(KO):
                pss = [psum2(0), psum2(1)]
                for t in range(KP):
                    w2 = (t * KO + mo) * 2
                    for c in range(NCH):
                        nc.tensor.matmul(
                            pss[c][:, 0:CH],
                            who_m[:, w2:w2 + 2, :],
                            A[c][:, 2 * t:2 * t + 2, :],
                            start=(t == 0), stop=(t == KP - 1),
                            perf_mode=DR)
                for c in range(NCH):
                    to = opool.tile([128, CH], F16, tag="preo", bufs=2,
                                    name="preo")
                    nc.vector.tensor_add(
                        to[:], pss[c][:, 0:CH],
                        outx[c][:, mo * CH:(mo + 1) * CH])
                    o = opool.tile([128, CH], BF16, tag="o", bufs=2, name="o")
                    nc.scalar.activation(o[:], to[:], AF.Sigmoid,
                                         bias=obc_t[:, mo:mo + 1], scale=INV)
                    eng = nc.sync if (mo + c) % 2 == 0 else nc.scalar
                    eng.dma_start(
                        outT[mo * 128:(mo + 1) * 128, c * CH:(c + 1) * CH],
                        o[:])

    _dedupe_ldweights(nc)
    nc.compile()
    return nc


_NC_CACHE = None


def _get_nc():
    global _NC_CACHE
    if _NC_CACHE is None:
        _NC_CACHE = _build_nc()
    return _NC_CACHE


def _make_bmask():
    m = np.zeros((128, 2 * CH), np.uint8)
    m[:_B1 - (_B1 // 128) * 128, 0:CH] = 1          # tile 5: parts < 43 tanh
    m[_B2 - (_B2 // 128) * 128:, CH:2 * CH] = 1     # tile 10: parts >= 86 relu
    return m


def _pack_dr(w_s, mt):
    """(NH, mt*128) k-major weights -> DoubleRow layout [128, KP*mt*2, 128].

    [p, (t*mt+m)*2+i, j] = w_s[(2t+i)*128 + p, m*128 + j]
    """
    fp8 = ml_dtypes.float8_e4m3
    w4 = w_s.reshape(KP, 2, 128, mt, 128)            # t, i, p, m, j
    return np.ascontiguousarray(
        w4.transpose(2, 0, 3, 1, 4).reshape(128, KP * mt * 2, 128)).astype(fp8)


def _prep_in_maps(inputs):
    bf = ml_dtypes.bfloat16
    x = np.asarray(inputs["inputs"], np.float32)
    hr = np.asarray(inputs["hidden_responses"], np.float32)[PERM]
    hb = np.asarray(inputs["hidden_biases"], np.float32)[PERM]
    orr = np.asarray(inputs["output_responses"], np.float32)
    ob = np.asarray(inputs["output_biases"], np.float32)

    wih_s = SCALE * (hr[:, None] *
                     np.asarray(inputs["input_to_hidden"], np.float32)[PERM]).T
    whh_s = SCALE * (hr[:, None] *
                     np.asarray(inputs["hidden_to_hidden"],
                                np.float32)[PERM][:, PERM]).T
    who_s = SCALE * (orr[:, None] *
                     np.asarray(inputs["hidden_to_output"],
                                np.float32)[:, PERM]).T
    wio_s = SCALE * (orr[:, None] *
                     np.asarray(inputs["input_to_output"], np.float32)).T

    def pack(w, ktiles):     # (ktiles*128, C) -> (128, ktiles*C)
        c = w.shape[1]
        return np.ascontiguousarray(
            w.reshape(ktiles, 128, c).transpose(1, 0, 2).reshape(128, ktiles * c))

    shared = {
        "wih": pack(wih_s, KI).astype(bf),
        "whh": _pack_dr(whh_s, KH),
        "who": _pack_dr(who_s, KO),
        "wio": pack(wio_s, KI).astype(bf),
        "hbc": np.ascontiguousarray(SCALE * hb.reshape(KH, 128).T),
        "obc": np.ascontiguousarray(ob.reshape(KO, 128).T),
        "bmask": _make_bmask(),
    }
    in_maps = []
    for c in range(N_CORES):
        m = dict(shared)
        m["xT"] = pack(np.ascontiguousarray(x[c * BL:(c + 1) * BL].T),
                       KI).astype(bf)
        in_maps.append(m)
    return in_maps


def _run(inputs, trace=False, tmpdir=None):
    nc = _get_nc()
    in_maps = _prep_in_maps(inputs)
    res = run_bass_kernel_spmd(nc, in_maps, core_ids=list(range(N_CORES)),
                               trace=trace, tmpdir=tmpdir)
    out = np.empty((B, NO), np.float32)
    for c in range(N_CORES):
        out[c * BL:(c + 1) * BL] = res.results[c]["outT"].T.astype(np.float32)
    return out, res


def kernel(**inputs) -> np.ndarray:
    out, _ = _run(inputs, trace=False)
    return out


if __name__ == "__main__":
    rng = np.random.default_rng(0)
    ins = {
        "inputs": rng.standard_normal((B, NI), dtype=np.float32),
        "input_to_hidden": rng.standard_normal((NH, NI), dtype=np.float32) * 0.02,
        "hidden_to_hidden": rng.standard_normal((NH, NH), dtype=np.float32) * 0.02,
        "output_to_hidden": rng.standard_normal((NH, NO), dtype=np.float32) * 0.02,
        "input_to_output": rng.standard_normal((NO, NI), dtype=np.float32) * 0.02,
        "hidden_to_output": rng.standard_normal((NO, NH), dtype=np.float32) * 0.02,
        "output_to_output": rng.standard_normal((NO, NO), dtype=np.float32) * 0.02,
        "hidden_responses": rng.standard_normal(NH, dtype=np.float32) * 0.1 + 1.0,
        "hidden_biases": rng.standard_normal(NH, dtype=np.float32) * 0.1,
        "output_responses": rng.standard_normal(NO, dtype=np.float32) * 0.1 + 1.0,
        "output_biases": rng.standard_normal(NO, dtype=np.float32) * 0.1,
    }
    out = kernel(**ins)
    print("kernel output", out.shape, out.dtype, out[:2, :4])


# revision 11
# speedup vs baseline: 1.7718x; 1.0074x over previous
"""Trainium2 Bass kernel for a 4-step differentiable recurrent net forward pass.

Reference computation (B=8192, NI=512, NH=2048, NO=512, 4 steps):
    activs = 0; outputs = 0
    repeat 4x:  pre = hr * (x @ Wih.T + activs @ Whh.T + outputs @ Woh.T) + hb
                activs = per_neuron_act(pre)        # tanh/sigmoid/relu by i%3
    out = sigmoid(or * (x @ Wio.T + outputs @ Woo.T + activs @ Who.T) + ob)

`outputs` is never written inside the loop, so the Woh/Woo terms vanish and
the x-projection P = hr*(x@Wih.T)+hb is loop-invariant (computed once).

Strategy: data-parallel on batch across 8 cores (1024 rows each). On-core
everything is feature-major (features on SBUF partitions, batch on the free
axis), so each matmul is W_tile.T @ X^T with stationary weights. The
recurrent Whh and the output Who matmuls (91% of tensor work) run in
fp8 e4m3 with perf_mode=DoubleRow: K=256 contraction per instruction at 2
MACs/PE-cycle. DoubleRow LDWEIGHTS (256 columns, no FWL) costs more than
the matmul itself, so both 512-row batch chunks are computed back-to-back
under one weight load: bass emits an InstLdweights per matmul, and a
post-build pass (_dedupe_ldweights) removes the redundant second load so
the non-self-loading second matmul reuses the array-resident weights.
Weights are scaled x32 before the fp8 cast (keeps them out of the
subnormal range); every PSUM eviction goes through the ACT engine with
scale=1/32 to compensate. Eviction temporaries are fp16 (the ACT engine
reads 2B/cycle/lane, so f32 sources run at half rate) and activations are
written as fp8 directly by the ACT engine. The input projections (x@Wih,
x@Wio) stay bf16 for accuracy. Host-side prep: hidden neurons are permuted
so the three activation groups are contiguous, hr/or are folded into the
weight matrices, weights are packed so each loads as one large contiguous
DMA, and hb/ob are applied as per-partition bias APs.
"""

import os

import numpy as np
import ml_dtypes

import concourse.bass as bass
import concourse.tile as tile
from concourse import bacc, mybir
from concourse.bass_utils import run_bass_kernel_spmd

B, NI, NH, NO = 8192, 512, 2048, 512
N_STEPS = 4
N_CORES = 8
BL = B // N_CORES          # batch rows per core
CH = 512                   # batch chunk (max moving free dim)
NCH = BL // CH             # 2 chunks per core
KI = NI // 128             # 4 k-tiles over inputs
KH = NH // 128             # 16 k/m-tiles over hidden
KP = KH // 2               # 8 DoubleRow k-pairs over hidden
KO = NO // 128             # 4 m-tiles over outputs

BF16 = mybir.dt.bfloat16
F16 = mybir.dt.float16
F32 = mybir.dt.float32
FP8 = mybir.dt.float8e4
AF = mybir.ActivationFunctionType
DR = mybir.MatmulPerfMode.DoubleRow

SCALE = 32.0               # fp8 weight pre-scale; undone by ACT scale=1/SCALE
INV = 1.0 / SCALE

# hidden neurons regrouped as [all tanh | all sigmoid | all relu]
_idx = np.arange(NH)
PERM = np.concatenate([_idx[_idx % 3 == 0], _idx[_idx % 3 == 1], _idx[_idx % 3 == 2]])
_B1 = int((_idx % 3 == 0).sum())           # 683
_B2 = _B1 + int((_idx % 3 == 1).sum())     # 1366

# per m-tile: the single activation function, or None for the two mixed tiles
_TILE_FUNC = []
for _m in range(KH):
    _lo, _hi = _m * 128, (_m + 1) * 128
    _fs = set()
    for _f, _a, _b in ((AF.Tanh, 0, _B1), (AF.Sigmoid, _B1, _B2), (AF.Relu, _B2, NH)):
        if max(_lo, _a) < min(_hi, _b):
            _fs.add(_f)
    _TILE_FUNC.append(_fs.pop() if len(_fs) == 1 else None)

# mixed tiles: (major_func applied everywhere, minor_func, mask column block)
# partition sub-ranges must be 32-aligned on TRN2, so the minority strip is
# fixed up with a full-tile ACT + copy_predicated against a {0,1} mask
_BOUNDARY = {
    _B1 // 128: (AF.Sigmoid, AF.Tanh, 0),    # tile 5: parts < 43 are tanh
    _B2 // 128: (AF.Sigmoid, AF.Relu, 1),    # tile 10: parts >= 86 are relu
}


def _dedupe_ldweights(nc):
    """Drop an InstLdweights that reloads exactly what the PE already holds.

    bass splits every matmul into InstLdweights + non-self-loading
    InstMatmult at build time; consecutive matmuls on the same stationary
    tile therefore carry a redundant (and expensive, for DoubleRow) reload.
    Only sync-free duplicates are removed, and any non-matmul PE
    instruction invalidates the tracked weights.
    """
    removed = 0
    for blk in nc.main_func.blocks:
        prev_key = None
        to_remove = []
        for i in blk.instructions:
            tn = type(i).__name__
            if tn == "InstLdweights":
                k = (repr(i.ins[0]), repr(i.perf_mode), repr(i.is_transpose),
                     repr(i.tile_position), repr(i.tile_size))
                si = i.sync_info
                clean = si is None or (len(si.on_wait) == 0
                                       and len(si.on_update) == 0)
                if k == prev_key and clean:
                    to_remove.append(i)
                else:
                    prev_key = k
            elif tn == "InstMatmult":
                continue
            elif getattr(i, "engine", None) == mybir.EngineType.PE:
                prev_key = None
        for i in to_remove:
            blk.instructions.remove(i)
        removed += len(to_remove)
    return removed


def _emit_hidden_act2(nc, ps, blk2, a_new, tmp_pool, bmask_t):
    """Evict a 2-m-tile pre-activation slab through the grouped activations.

    ps:    SBUF AP (128, 2*CH) fp16/bf16 holding SCALE*pre for m-tiles
           2*blk2, 2*blk2+1
    a_new: SBUF tile (128, KH, CH) fp8, m-tile m lives at [:, m, :]
    """
    mloc = 0
    while mloc < 2:
        m = 2 * blk2 + mloc
        if m in _BOUNDARY:
            major, minor, mb = _BOUNDARY[m]
            nc.scalar.activation(
                a_new[:, m, :], ps[:, mloc * CH:(mloc + 1) * CH], major,
                scale=INV)
            t = tmp_pool.tile([128, CH], FP8, tag="btmp", bufs=2, name="btmp")
            nc.scalar.activation(t[:], ps[:, mloc * CH:(mloc + 1) * CH], minor,
                                 scale=INV)
            nc.vector.copy_predicated(
                a_new[:, m, :], bmask_t[:, mb * CH:(mb + 1) * CH], t[:])
            mloc += 1
            continue
        func = _TILE_FUNC[m]
        end = mloc + 1
        while end < 2 and _TILE_FUNC[2 * blk2 + end] == func:
            end += 1
        nc.scalar.activation(
            a_new[:, 2 * blk2 + mloc:2 * blk2 + end, :],
            ps[:, mloc * CH:end * CH], func, scale=INV)
        mloc = end


def _build_nc():
    nc = bacc.Bacc("TRN2", target_bir_lowering=False, debug=False,
                   num_devices=N_CORES, dynamic_dma_scratch_size=2048)

    # all operands host-packed so DMA arrival order matches compute order:
    # wih: m-block-major [p, (b*KI+k)*256 + j] = Wih_s[k*128+p, b*256+j]
    #      (the P phase consumes 2-m-tile blocks; one 256KB DMA per block)
    # x:   chunk-major [p, (c*KI+k)*CH + col] = xT[k*128+p, c*CH+col]
    # wio: [p, k*cols + c] = W[k*128 + p, c] for k-tile k
    # whh (fp8): [p, (t*16+m)*2+i, j] = Whh_s[(2t+i)*128+p, m*128+j]
    # who (fp8): [p, (t*4+mo)*2+i, j] = Who_s[(2t+i)*128+p, mo*128+j]
    xT = nc.dram_tensor("xT", [128, KI * BL], BF16, kind="ExternalInput").ap()
    wih = nc.dram_tensor("wih", [128, KI * NH], BF16, kind="ExternalInput").ap()
    whh = nc.dram_tensor("whh", [128, KP * KH * 2, 128], FP8,
                         kind="ExternalInput").ap()
    who = nc.dram_tensor("who", [128, KP * KO * 2, 128], FP8,
                         kind="ExternalInput").ap()
    wio = nc.dram_tensor("wio", [128, KI * NO], BF16, kind="ExternalInput").ap()
    hbc = nc.dram_tensor("hbc", [128, KH], F32, kind="ExternalInput").ap()
    obc = nc.dram_tensor("obc", [128, KO], F32, kind="ExternalInput").ap()
    bmask = nc.dram_tensor("bmask", [128, 2 * CH], mybir.dt.uint8,
                           kind="ExternalInput").ap()
    outT = nc.dram_tensor("outT", [NO, BL], BF16, kind="ExternalOutput").ap()

    with tile.TileContext(nc) as tc:
        with tc.tile_pool(name="w", bufs=1) as wpool, \
             tc.tile_pool(name="act", bufs=1) as apool, \
             tc.tile_pool(name="ps", bufs=2, space="PSUM") as pspool, \
             tc.tile_pool(name="out", bufs=4) as opool:

            # ---- stage inputs. wih lands as one 256KB DMA per 2-m-tile
            # block (sync queue) in the exact order the P phase consumes
            # them; x lands chunk-major on the scalar queue. The big fp8
            # loads follow on sync/scalar, with who on the vector queue.
            wih_m = wpool.tile([128, KI * NH], BF16, tag="wih", name="wihm")
            x_m = wpool.tile([128, KI * BL], BF16, tag="x", name="xm")
            nc.sync.dma_start(wih_m[:, 0:KI * 256], wih[:, 0:KI * 256])
            nc.scalar.dma_start(x_m[:, 0:CH], xT[:, 0:CH])
            nc.sync.dma_start(wih_m[:, KI * 256:2 * KI * 256],
                              wih[:, KI * 256:2 * KI * 256])
            nc.scalar.dma_start(x_m[:, CH:2 * CH], xT[:, CH:2 * CH])
            for b in range(2, 8):
                nc.sync.dma_start(
                    wih_m[:, b * KI * 256:(b + 1) * KI * 256],
                    wih[:, b * KI * 256:(b + 1) * KI * 256])
            nc.scalar.dma_start(x_m[:, 2 * CH:4 * CH], xT[:, 2 * CH:4 * CH])
            hbc_t = wpool.tile([128, KH], F32, tag="hbc")
            nc.scalar.dma_start(hbc_t[:], hbc[:])
            obc_t = wpool.tile([128, KO], F32, tag="obc")
            nc.scalar.dma_start(obc_t[:], obc[:])
            bmask_t = wpool.tile([128, 2 * CH], mybir.dt.uint8, tag="bmask")
            nc.scalar.dma_start(bmask_t[:], bmask[:])
            nc.scalar.dma_start(x_m[:, 4 * CH:6 * CH], xT[:, 4 * CH:6 * CH])
            nc.scalar.dma_start(x_m[:, 6 * CH:8 * CH], xT[:, 6 * CH:8 * CH])
            wio_m = wpool.tile([128, KI * NO], BF16, tag="wio", name="wiom")
            nc.scalar.dma_start(wio_m[:], wio[:])
            # whh fp8: 4MB total as 4x 1MB DMAs over sync+scalar queues
            whh_m = wpool.tile([128, KP * KH * 2, 128], FP8, tag="whh",
                               name="whhm")
            for J in range(4):
                eng = nc.sync if J % 2 == 0 else nc.scalar
                eng.dma_start(whh_m[:, J * 64:(J + 1) * 64, :],
                              whh[:, J * 64:(J + 1) * 64, :])
            who_m = wpool.tile([128, KP * KO * 2, 128], FP8, tag="who",
                               name="whom")
            nc.scalar.dma_start(who_m[:], who[:])

            def xT_ap(k, c):
                lo = (c * KI + k) * CH
                return x_m[:, lo:lo + CH]

            def wih_ap(k, m):
                lo = ((m // 2) * KI + k) * 256 + (m % 2) * 128
                return wih_m[:, lo:lo + 128]

            wio_t = [wio_m[:, k * NO:(k + 1) * NO] for k in range(KI)]

            def psum2(i):
                # two 2-bank accumulators live at once (one per chunk, or
                # pipelined across 2-m-tile blocks); bufs=2 each fills PSUM
                return pspool.tile([128, 2 * CH], F32,
                                   tag=("psA" if i % 2 == 0 else "psB"),
                                   bufs=2, name="psb")

            # ---- per-chunk x-projection P (= SCALE*(hr*(x@Wih.T)+hb), bf16)
            # and first-step activations, in 2-m-tile blocks ----
            P = {}
            A = {}
            for c in range(NCH):
                P[c] = apool.tile([128, KH * CH], BF16, tag=f"P{c}",
                                  name=f"P{c}")
                a1 = apool.tile([128, KH, CH], FP8, tag=f"A{c}", bufs=2,
                                name=f"A1c{c}")
                for blk in range(8):
                    ps = psum2(blk)
                    for k in range(KI):
                        for mloc in range(2):
                            m = 2 * blk + mloc
                            nc.tensor.matmul(
                                ps[:, mloc * CH:(mloc + 1) * CH],
                                wih_ap(k, m),
                                xT_ap(k, c),
                                start=(k == 0), stop=(k == KI - 1))
                    for mloc in range(2):
                        m = 2 * blk + mloc
                        nc.vector.tensor_scalar_add(
                            P[c][:, m * CH:(m + 1) * CH],
                            ps[:, mloc * CH:(mloc + 1) * CH],
                            hbc_t[:, m:m + 1])
                    # A1 = act(P/SCALE) straight from SBUF — frees the PSUM
                    # slot as soon as the adds have read it
                    _emit_hidden_act2(nc, P[c][:, 2 * blk * CH:(2 * blk + 2) * CH],
                                      blk, a1, opool, bmask_t)
                A[c] = a1

            # ---- whh-independent output x-projection (fills the window
            # while the 4MB whh load is still in flight); holds SCALE*or*
            # (x@Wio.T) ----
            outx = {}
            for c in range(NCH):
                outx[c] = apool.tile([128, KO * CH], BF16, tag=f"outx{c}",
                                     name=f"outx{c}")
                for half in range(2):
                    ps = psum2(half)
                    for k in range(KI):
                        for mloc in range(2):
                            mo = 2 * half + mloc
                            nc.tensor.matmul(
                                ps[:, mloc * CH:(mloc + 1) * CH],
                                wio_t[k][:, mo * 128:(mo + 1) * 128],
                                xT_ap(k, c),
                                start=(k == 0), stop=(k == KI - 1))
                    nc.vector.tensor_copy(
                        outx[c][:, 2 * half * CH:(2 * half + 2) * CH], ps[:])

            # ---- recurrent steps 2..4: fp8 DoubleRow, both chunks computed
            # back-to-back per weight load (the dedupe pass removes the
            # second LDWEIGHTS) ----
            def hh_step_fused(s):
                a_new = [apool.tile([128, KH, CH], FP8, tag=f"A{c}", bufs=2,
                                    name=f"A{s + 2}c{c}") for c in range(NCH)]
                for blk in range(8):
                    pss = [psum2(0), psum2(1)]
                    for t in range(KP):
                        for mloc in range(2):
                            m = 2 * blk + mloc
                            w2 = (t * KH + m) * 2
                            for c in range(NCH):
                                nc.tensor.matmul(
                                    pss[c][:, mloc * CH:(mloc + 1) * CH],
                                    whh_m[:, w2:w2 + 2, :],
                                    A[c][:, 2 * t:2 * t + 2, :],
                                    start=(t == 0), stop=(t == KP - 1),
                                    perf_mode=DR)
                    for c in range(NCH):
                        # pre = psum + P into an fp16 SBUF temp: a single
                        # PSUM read frees the bank, and the ACT engine reads
                        # 16-bit sources at full rate
                        tmp = opool.tile([128, 2 * CH], F16, tag="pre",
                                         bufs=4, name="pre")
                        nc.vector.tensor_add(
                            tmp[:], pss[c][:],
                            P[c][:, 2 * blk * CH:(2 * blk + 2) * CH])
                        _emit_hidden_act2(nc, tmp, blk, a_new[c], opool,
                                          bmask_t)
                for c in range(NCH):
                    A[c] = a_new[c]

            for s in range(N_STEPS - 1):
                hh_step_fused(s)

            # ---- output layer (fp8 DoubleRow, same weight reuse) ----
            for mo in range(KO):
                pss = [psum2(0), psum2(1)]
                for t in range(KP):
                    w2 = (t * KO + mo) * 2
                    for c in range(NCH):
                        nc.tensor.matmul(
                            pss[c][:, 0:CH],
                            who_m[:, w2:w2 + 2, :],
                            A[c][:, 2 * t:2 * t + 2, :],
                            start=(t == 0), stop=(t == KP - 1),
                            perf_mode=DR)
                # half-tile evictions keep the post-matmul drain short
                for c in range(NCH):
                    for h in range(2):
                        lo, hi = h * (CH // 2), (h + 1) * (CH // 2)
                        to = opool.tile([128, CH // 2], F16, tag="preo",
                                        bufs=4, name="preo")
                        nc.vector.tensor_add(
                            to[:], pss[c][:, lo:hi],
                            outx[c][:, mo * CH + lo:mo * CH + hi])
                        o = opool.tile([128, CH // 2], BF16, tag="o", bufs=4,
                                       name="o")
                        nc.scalar.activation(o[:], to[:], AF.Sigmoid,
                                             bias=obc_t[:, mo:mo + 1],
                                             scale=INV)
                        eng = nc.sync if (c + h) % 2 == 0 else nc.scalar
                        eng.dma_start(
                            outT[mo * 128:(mo + 1) * 128,
                                 c * CH + lo:c * CH + hi],
                            o[:])

    _dedupe_ldweights(nc)
    nc.compile()
    return nc


_NC_CACHE = None


def _get_nc():
    global _NC_CACHE
    if _NC_CACHE is None:
        _NC_CACHE = _build_nc()
    return _NC_CACHE


def _make_bmask():
    m = np.zeros((128, 2 * CH), np.uint8)
    m[:_B1 - (_B1 // 128) * 128, 0:CH] = 1          # tile 5: parts < 43 tanh
    m[_B2 - (_B2 // 128) * 128:, CH:2 * CH] = 1     # tile 10: parts >= 86 relu
    return m


def _pack_dr(w_s, mt):
    """(NH, mt*128) k-major weights -> DoubleRow layout [128, KP*mt*2, 128].

    [p, (t*mt+m)*2+i, j] = w_s[(2t+i)*128 + p, m*128 + j]
    """
    fp8 = ml_dtypes.float8_e4m3
    w4 = w_s.reshape(KP, 2, 128, mt, 128)            # t, i, p, m, j
    return np.ascontiguousarray(
        w4.transpose(2, 0, 3, 1, 4).reshape(128, KP * mt * 2, 128)).astype(fp8)


def _prep_in_maps(inputs):
    bf = ml_dtypes.bfloat16
    x = np.asarray(inputs["inputs"], np.float32)
    hr = np.asarray(inputs["hidden_responses"], np.float32)[PERM]
    hb = np.asarray(inputs["hidden_biases"], np.float32)[PERM]
    orr = np.asarray(inputs["output_responses"], np.float32)
    ob = np.asarray(inputs["output_biases"], np.float32)

    wih_s = SCALE * (hr[:, None] *
                     np.asarray(inputs["input_to_hidden"], np.float32)[PERM]).T
    whh_s = SCALE * (hr[:, None] *
                     np.asarray(inputs["hidden_to_hidden"],
                                np.float32)[PERM][:, PERM]).T
    who_s = SCALE * (orr[:, None] *
                     np.asarray(inputs["hidden_to_output"],
                                np.float32)[:, PERM]).T
    wio_s = SCALE * (orr[:, None] *
                     np.asarray(inputs["input_to_output"], np.float32)).T

    def pack(w, ktiles):     # (ktiles*128, C) -> (128, ktiles*C)
        c = w.shape[1]
        return np.ascontiguousarray(
            w.reshape(ktiles, 128, c).transpose(1, 0, 2).reshape(128, ktiles * c))

    # wih m-block-major: [p, (b*KI+k)*256 + j] = wih_s[k*128+p, b*256+j]
    wih_p = np.ascontiguousarray(
        wih_s.reshape(KI, 128, 8, 256).transpose(1, 2, 0, 3)
        .reshape(128, KI * NH))

    shared = {
        "wih": wih_p.astype(bf),
        "whh": _pack_dr(whh_s, KH),
        "who": _pack_dr(who_s, KO),
        "wio": pack(wio_s, KI).astype(bf),
        "hbc": np.ascontiguousarray(SCALE * hb.reshape(KH, 128).T),
        "obc": np.ascontiguousarray(ob.reshape(KO, 128).T),
        "bmask": _make_bmask(),
    }
    in_maps = []
    for c in range(N_CORES):
        m = dict(shared)
        # x chunk-major: [p, (ch*KI+k)*CH + col] = x.T[k*128+p, ch*CH+col]
        xc = np.ascontiguousarray(x[c * BL:(c + 1) * BL].T)     # (NI, BL)
        m["xT"] = np.ascontiguousarray(
            xc.reshape(KI, 128, NCH, CH).transpose(1, 2, 0, 3)
            .reshape(128, KI * BL)).astype(bf)
        in_maps.append(m)
    return in_maps


def _run(inputs, trace=False, tmpdir=None):
    nc = _get_nc()
    in_maps = _prep_in_maps(inputs)
    res = run_bass_kernel_spmd(nc, in_maps, core_ids=list(range(N_CORES)),
                               trace=trace, tmpdir=tmpdir)
    out = np.empty((B, NO), np.float32)
    for c in range(N_CORES):
        out[c * BL:(c + 1) * BL] = res.results[c]["outT"].T.astype(np.float32)
    return out, res


def kernel(**inputs) -> np.ndarray:
    out, _ = _run(inputs, trace=False)
    return out


if __name__ == "__main__":
    rng = np.random.default_rng(0)
    ins = {
        "inputs": rng.standard_normal((B, NI), dtype=np.float32),
        "input_to_hidden": rng.standard_normal((NH, NI), dtype=np.float32) * 0.02,
        "hidden_to_hidden": rng.standard_normal((NH, NH), dtype=np.float32) * 0.02,
        "output_to_hidden": rng.standard_normal((NH, NO), dtype=np.float32) * 0.02,
        "input_to_output": rng.standard_normal((NO, NI), dtype=np.float32) * 0.02,
        "hidden_to_output": rng.standard_normal((NO, NH), dtype=np.float32) * 0.02,
        "output_to_output": rng.standard_normal((NO, NO), dtype=np.float32) * 0.02,
        "hidden_responses": rng.standard_normal(NH, dtype=np.float32) * 0.1 + 1.0,
        "hidden_biases": rng.standard_normal(NH, dtype=np.float32) * 0.1,
        "output_responses": rng.standard_normal(NO, dtype=np.float32) * 0.1 + 1.0,
        "output_biases": rng.standard_normal(NO, dtype=np.float32) * 0.1,
    }
    out = kernel(**ins)
    print("kernel output", out.shape, out.dtype, out[:2, :4])


# revision 17
# speedup vs baseline: 1.8640x; 1.0521x over previous
"""Trainium2 Bass kernel for a 4-step differentiable recurrent net forward pass.

Reference computation (B=8192, NI=512, NH=2048, NO=512, 4 steps):
    activs = 0; outputs = 0
    repeat 4x:  pre = hr * (x @ Wih.T + activs @ Whh.T + outputs @ Woh.T) + hb
                activs = per_neuron_act(pre)        # tanh/sigmoid/relu by i%3
    out = sigmoid(or * (x @ Wio.T + outputs @ Woo.T + activs @ Who.T) + ob)

`outputs` is never written inside the loop, so the Woh/Woo terms vanish and
the x-projection P = hr*(x@Wih.T)+hb is loop-invariant (computed once).

Strategy: data-parallel on batch across 8 cores (1024 rows each). On-core
everything is feature-major (features on SBUF partitions, batch on the free
axis), so each matmul is W_tile.T @ X^T with stationary weights. The
recurrent Whh and the output Who matmuls (91% of tensor work) run in
fp8 e4m3 with perf_mode=DoubleRow: K=256 contraction per instruction at 2
MACs/PE-cycle. DoubleRow LDWEIGHTS (256 columns, no FWL) costs more than
the matmul itself, so both 512-row batch chunks are computed back-to-back
under one weight load: bass emits an InstLdweights per matmul, and a
post-build pass (_dedupe_ldweights) removes the redundant second load so
the non-self-loading second matmul reuses the array-resident weights.
Weights are scaled x32 before the fp8 cast (keeps them out of the
subnormal range); every PSUM eviction goes through the ACT engine with
scale=1/32 to compensate. Eviction temporaries are fp16 (the ACT engine
reads 2B/cycle/lane, so f32 sources run at half rate) and activations are
written as fp8 directly by the ACT engine. The input projections (x@Wih,
x@Wio) are fp8 DoubleRow as well — simulated end-to-end rel err 1.27e-2
vs the 2e-2 budget. Host-side prep: hidden neurons are permuted
so the three activation groups are contiguous, hr/or are folded into the
weight matrices, weights are packed so each loads as one large contiguous
DMA, and hb/ob are applied as per-partition bias APs.
"""

import os

import numpy as np
import ml_dtypes

import concourse.bass as bass
import concourse.tile as tile
from concourse import bacc, mybir
from concourse.bass_utils import run_bass_kernel_spmd

B, NI, NH, NO = 8192, 512, 2048, 512
N_STEPS = 4
N_CORES = 8
BL = B // N_CORES          # batch rows per core
CH = 512                   # batch chunk (max moving free dim)
NCH = BL // CH             # 2 chunks per core
KI = NI // 128             # 4 k-tiles over inputs
KH = NH // 128             # 16 k/m-tiles over hidden
KP = KH // 2               # 8 DoubleRow k-pairs over hidden
KO = NO // 128             # 4 m-tiles over outputs

BF16 = mybir.dt.bfloat16
F16 = mybir.dt.float16
F32 = mybir.dt.float32
FP8 = mybir.dt.float8e4
AF = mybir.ActivationFunctionType
DR = mybir.MatmulPerfMode.DoubleRow

SCALE = 32.0               # fp8 weight pre-scale; undone by ACT scale=1/SCALE
INV = 1.0 / SCALE

# hidden neurons regrouped as [all tanh | all sigmoid | all relu]
_idx = np.arange(NH)
PERM = np.concatenate([_idx[_idx % 3 == 0], _idx[_idx % 3 == 1], _idx[_idx % 3 == 2]])
_B1 = int((_idx % 3 == 0).sum())           # 683
_B2 = _B1 + int((_idx % 3 == 1).sum())     # 1366

# per m-tile: the single activation function, or None for the two mixed tiles
_TILE_FUNC = []
for _m in range(KH):
    _lo, _hi = _m * 128, (_m + 1) * 128
    _fs = set()
    for _f, _a, _b in ((AF.Tanh, 0, _B1), (AF.Sigmoid, _B1, _B2), (AF.Relu, _B2, NH)):
        if max(_lo, _a) < min(_hi, _b):
            _fs.add(_f)
    _TILE_FUNC.append(_fs.pop() if len(_fs) == 1 else None)

# mixed tiles: (major_func applied everywhere, minor_func, mask column block)
# partition sub-ranges must be 32-aligned on TRN2, so the minority strip is
# fixed up with a full-tile ACT + copy_predicated against a {0,1} mask
_BOUNDARY = {
    _B1 // 128: (AF.Sigmoid, AF.Tanh, 0),    # tile 5: parts < 43 are tanh
    _B2 // 128: (AF.Sigmoid, AF.Relu, 1),    # tile 10: parts >= 86 are relu
}


def _dedupe_ldweights(nc):
    """Drop an InstLdweights that reloads exactly what the PE already holds.

    bass splits every matmul into InstLdweights + non-self-loading
    InstMatmult at build time; consecutive matmuls on the same stationary
    tile therefore carry a redundant (and expensive, for DoubleRow) reload.
    Only sync-free duplicates are removed, and any non-matmul PE
    instruction invalidates the tracked weights.
    """
    removed = 0
    for blk in nc.main_func.blocks:
        prev_key = None
        to_remove = []
        for i in blk.instructions:
            tn = type(i).__name__
            if tn == "InstLdweights":
                k = (repr(i.ins[0]), repr(i.perf_mode), repr(i.is_transpose),
                     repr(i.tile_position), repr(i.tile_size))
                si = i.sync_info
                clean = si is None or (len(si.on_wait) == 0
                                       and len(si.on_update) == 0)
                if k == prev_key and clean:
                    to_remove.append(i)
                else:
                    prev_key = k
            elif tn == "InstMatmult":
                continue
            elif getattr(i, "engine", None) == mybir.EngineType.PE:
                prev_key = None
        for i in to_remove:
            blk.instructions.remove(i)
        removed += len(to_remove)
    return removed


def _emit_hidden_act2(nc, ps, blk2, a_new, tmp_pool, bmask_t):
    """Evict a 2-m-tile pre-activation slab through the grouped activations.

    ps:    SBUF AP (128, 2*CH) fp16/bf16 holding SCALE*pre for m-tiles
           2*blk2, 2*blk2+1
    a_new: SBUF tile (128, KH, CH) fp8, m-tile m lives at [:, m, :]
    """
    mloc = 0
    while mloc < 2:
        m = 2 * blk2 + mloc
        if m in _BOUNDARY:
            major, minor, mb = _BOUNDARY[m]
            nc.scalar.activation(
                a_new[:, m, :], ps[:, mloc * CH:(mloc + 1) * CH], major,
                scale=INV)
            t = tmp_pool.tile([128, CH], FP8, tag="btmp", bufs=2, name="btmp")
            nc.scalar.activation(t[:], ps[:, mloc * CH:(mloc + 1) * CH], minor,
                                 scale=INV)
            nc.vector.copy_predicated(
                a_new[:, m, :], bmask_t[:, mb * CH:(mb + 1) * CH], t[:])
            mloc += 1
            continue
        func = _TILE_FUNC[m]
        end = mloc + 1
        while end < 2 and _TILE_FUNC[2 * blk2 + end] == func:
            end += 1
        nc.scalar.activation(
            a_new[:, 2 * blk2 + mloc:2 * blk2 + end, :],
            ps[:, mloc * CH:end * CH], func, scale=INV)
        mloc = end


def _build_nc():
    nc = bacc.Bacc("TRN2", target_bir_lowering=False, debug=False,
                   num_devices=N_CORES, dynamic_dma_scratch_size=2048)

    # all operands fp8, host-packed for DoubleRow APs with DMA arrival order
    # matching compute order:
    # x:   [p, (c*2+t)*2+i, col] = x.T[(2t+i)*128+p, c*CH+col]
    # wih: m-block-major [p, ((b*2+t)*2+mloc)*2+i, j]
    #        = Wih_s[(2t+i)*128+p, (2b+mloc)*128+j]
    # wio: [p, (t*4+mo)*2+i, j] = Wio_s[(2t+i)*128+p, mo*128+j]
    # whh: [p, (t*16+m)*2+i, j] = Whh_s[(2t+i)*128+p, m*128+j]
    # who: [p, (t*4+mo)*2+i, j] = Who_s[(2t+i)*128+p, mo*128+j]
    xT = nc.dram_tensor("xT", [128, NCH * 4, CH], FP8,
                        kind="ExternalInput").ap()
    wih = nc.dram_tensor("wih", [128, 8 * 4 * 2, 128], FP8,
                         kind="ExternalInput").ap()
    whh = nc.dram_tensor("whh", [128, KP * KH * 2, 128], FP8,
                         kind="ExternalInput").ap()
    who = nc.dram_tensor("who", [128, KP * KO * 2, 128], FP8,
                         kind="ExternalInput").ap()
    wio = nc.dram_tensor("wio", [128, 2 * KO * 2, 128], FP8,
                         kind="ExternalInput").ap()
    hbc = nc.dram_tensor("hbc", [128, KH], F32, kind="ExternalInput").ap()
    obc = nc.dram_tensor("obc", [128, KO], F32, kind="ExternalInput").ap()
    bmask = nc.dram_tensor("bmask", [128, 2 * CH], mybir.dt.uint8,
                           kind="ExternalInput").ap()
    outT = nc.dram_tensor("outT", [NO, BL], BF16, kind="ExternalOutput").ap()

    with tile.TileContext(nc) as tc:
        with tc.tile_pool(name="w", bufs=1) as wpool, \
             tc.tile_pool(name="act", bufs=1) as apool, \
             tc.tile_pool(name="ps", bufs=2, space="PSUM") as pspool, \
             tc.tile_pool(name="out", bufs=4) as opool:

            # ---- stage inputs. wih lands as one 128KB DMA per 2-m-tile
            # block (sync queue) in the exact order the P phase consumes
            # them; x lands chunk-major on the scalar queue; the 4MB whh
            # follows split across both queues.
            wih_m = wpool.tile([128, 8 * 4 * 2, 128], FP8, tag="wih",
                               name="wihm")
            x_m = wpool.tile([128, NCH * 4, CH], FP8, tag="x", name="xm")
            nc.sync.dma_start(wih_m[:, 0:8, :], wih[:, 0:8, :])
            nc.scalar.dma_start(x_m[:, 0:4, :], xT[:, 0:4, :])
            for b in range(1, 8):
                nc.sync.dma_start(wih_m[:, b * 8:(b + 1) * 8, :],
                                  wih[:, b * 8:(b + 1) * 8, :])
            nc.scalar.dma_start(x_m[:, 4:8, :], xT[:, 4:8, :])
            hbc_t = wpool.tile([128, KH], F32, tag="hbc")
            nc.scalar.dma_start(hbc_t[:], hbc[:])
            obc_t = wpool.tile([128, KO], F32, tag="obc")
            nc.scalar.dma_start(obc_t[:], obc[:])
            bmask_t = wpool.tile([128, 2 * CH], mybir.dt.uint8, tag="bmask")
            nc.scalar.dma_start(bmask_t[:], bmask[:])
            wio_m = wpool.tile([128, 2 * KO * 2, 128], FP8, tag="wio",
                               name="wiom")
            nc.scalar.dma_start(wio_m[:], wio[:])
            # whh fp8: 4MB total as 4x 1MB DMAs over sync+scalar queues
            whh_m = wpool.tile([128, KP * KH * 2, 128], FP8, tag="whh",
                               name="whhm")
            for J in range(4):
                eng = nc.sync if J % 2 == 0 else nc.scalar
                eng.dma_start(whh_m[:, J * 64:(J + 1) * 64, :],
                              whh[:, J * 64:(J + 1) * 64, :])
            who_m = wpool.tile([128, KP * KO * 2, 128], FP8, tag="who",
                               name="whom")
            nc.scalar.dma_start(who_m[:], who[:])

            def x8_ap(t, c):
                lo = (c * 2 + t) * 2
                return x_m[:, lo:lo + 2, :]

            def wih8_ap(t, m):
                lo = ((m // 2 * 2 + t) * 2 + m % 2) * 2
                return wih_m[:, lo:lo + 2, :]

            def wio8_ap(t, mo):
                lo = (t * KO + mo) * 2
                return wio_m[:, lo:lo + 2, :]

            def psum2(i):
                # two 2-bank accumulators live at once (one per chunk, or
                # pipelined across 2-m-tile blocks); bufs=2 each fills PSUM
                return pspool.tile([128, 2 * CH], F32,
                                   tag=("psA" if i % 2 == 0 else "psB"),
                                   bufs=2, name="psb")

            # ---- x-projection P (= SCALE*(hr*(x@Wih.T)+hb), bf16) and
            # first-step activations: fp8 DoubleRow (K=512 = 2 pairs), both
            # chunks per weight load, 2-m-tile blocks ----
            P = {}
            A = {}
            for c in range(NCH):
                P[c] = apool.tile([128, KH * CH], BF16, tag=f"P{c}",
                                  name=f"P{c}")
                A[c] = apool.tile([128, KH, CH], FP8, tag=f"A{c}", bufs=2,
                                  name=f"A1c{c}")
            for blk in range(8):
                pss = [psum2(0), psum2(1)]
                for t in range(2):
                    for mloc in range(2):
                        m = 2 * blk + mloc
                        for c in range(NCH):
                            nc.tensor.matmul(
                                pss[c][:, mloc * CH:(mloc + 1) * CH],
                                wih8_ap(t, m), x8_ap(t, c),
                                start=(t == 0), stop=(t == 1), perf_mode=DR)
                for c in range(NCH):
                    for mloc in range(2):
                        m = 2 * blk + mloc
                        nc.vector.tensor_scalar_add(
                            P[c][:, m * CH:(m + 1) * CH],
                            pss[c][:, mloc * CH:(mloc + 1) * CH],
                            hbc_t[:, m:m + 1])
                    # A1 = act(P/SCALE) straight from SBUF — frees the PSUM
                    # slot as soon as the adds have read it
                    _emit_hidden_act2(
                        nc, P[c][:, 2 * blk * CH:(2 * blk + 2) * CH],
                        blk, A[c], opool, bmask_t)

            # ---- whh-independent output x-projection (fills the window
            # while the 4MB whh load is still in flight); holds SCALE*or*
            # (x@Wio.T) ----
            outx = {}
            for c in range(NCH):
                outx[c] = apool.tile([128, KO * CH], BF16, tag=f"outx{c}",
                                     name=f"outx{c}")
            for half in range(2):
                pss = [psum2(0), psum2(1)]
                for t in range(2):
                    for mloc in range(2):
                        mo = 2 * half + mloc
                        for c in range(NCH):
                            nc.tensor.matmul(
                                pss[c][:, mloc * CH:(mloc + 1) * CH],
                                wio8_ap(t, mo), x8_ap(t, c),
                                start=(t == 0), stop=(t == 1), perf_mode=DR)
                for c in range(NCH):
                    nc.vector.tensor_copy(
                        outx[c][:, 2 * half * CH:(2 * half + 2) * CH],
                        pss[c][:])

            # ---- recurrent steps 2..4: fp8 DoubleRow, both chunks computed
            # back-to-back per weight load (the dedupe pass removes the
            # second LDWEIGHTS) ----
            def hh_step_fused(s):
                a_new = [apool.tile([128, KH, CH], FP8, tag=f"A{c}", bufs=2,
                                    name=f"A{s + 2}c{c}") for c in range(NCH)]
                for blk in range(8):
                    pss = [psum2(0), psum2(1)]
                    for t in range(KP):
                        for mloc in range(2):
                            m = 2 * blk + mloc
                            w2 = (t * KH + m) * 2
                            for c in range(NCH):
                                nc.tensor.matmul(
                                    pss[c][:, mloc * CH:(mloc + 1) * CH],
                                    whh_m[:, w2:w2 + 2, :],
                                    A[c][:, 2 * t:2 * t + 2, :],
                                    start=(t == 0), stop=(t == KP - 1),
                                    perf_mode=DR)
                    for c in range(NCH):
                        # pre = psum + P into an fp16 SBUF temp: a single
                        # PSUM read frees the bank, and the ACT engine reads
                        # 16-bit sources at full rate
                        tmp = opool.tile([128, 2 * CH], F16, tag="pre",
                                         bufs=4, name="pre")
                        nc.vector.tensor_add(
                            tmp[:], pss[c][:],
                            P[c][:, 2 * blk * CH:(2 * blk + 2) * CH])
                        _emit_hidden_act2(nc, tmp, blk, a_new[c], opool,
                                          bmask_t)
                for c in range(NCH):
                    A[c] = a_new[c]

            for s in range(N_STEPS - 1):
                hh_step_fused(s)

            # ---- output layer (fp8 DoubleRow, same weight reuse) ----
            for mo in range(KO):
                pss = [psum2(0), psum2(1)]
                for t in range(KP):
                    w2 = (t * KO + mo) * 2
                    for c in range(NCH):
                        nc.tensor.matmul(
                            pss[c][:, 0:CH],
                            who_m[:, w2:w2 + 2, :],
                            A[c][:, 2 * t:2 * t + 2, :],
                            start=(t == 0), stop=(t == KP - 1),
                            perf_mode=DR)
                # half-tile evictions keep the post-matmul drain short
                for c in range(NCH):
                    for h in range(2):
                        lo, hi = h * (CH // 2), (h + 1) * (CH // 2)
                        to = opool.tile([128, CH // 2], F16, tag="preo",
                                        bufs=4, name="preo")
                        nc.vector.tensor_add(
                            to[:], pss[c][:, lo:hi],
                            outx[c][:, mo * CH + lo:mo * CH + hi])
                        o = opool.tile([128, CH // 2], BF16, tag="o", bufs=4,
                                       name="o")
                        nc.scalar.activation(o[:], to[:], AF.Sigmoid,
                                             bias=obc_t[:, mo:mo + 1],
                                             scale=INV)
                        eng = nc.sync if (c + h) % 2 == 0 else nc.scalar
                        eng.dma_start(
                            outT[mo * 128:(mo + 1) * 128,
                                 c * CH + lo:c * CH + hi],
                            o[:])

    _dedupe_ldweights(nc)
    nc.compile()
    return nc


_NC_CACHE = None


def _get_nc():
    global _NC_CACHE
    if _NC_CACHE is None:
        _NC_CACHE = _build_nc()
    return _NC_CACHE


def _make_bmask():
    m = np.zeros((128, 2 * CH), np.uint8)
    m[:_B1 - (_B1 // 128) * 128, 0:CH] = 1          # tile 5: parts < 43 tanh
    m[_B2 - (_B2 // 128) * 128:, CH:2 * CH] = 1     # tile 10: parts >= 86 relu
    return m


def _pack_dr(w_s, kp, mt):
    """(kp*256, mt*128) k-major weights -> DoubleRow layout
    [128, kp*mt*2, 128]: [p, (t*mt+m)*2+i, j] = w_s[(2t+i)*128+p, m*128+j]
    """
    fp8 = ml_dtypes.float8_e4m3
    w4 = w_s.reshape(kp, 2, 128, mt, 128)            # t, i, p, m, j
    return np.ascontiguousarray(
        w4.transpose(2, 0, 3, 1, 4).reshape(128, kp * mt * 2, 128)).astype(fp8)


def _prep_in_maps(inputs):
    bf = ml_dtypes.bfloat16
    x = np.asarray(inputs["inputs"], np.float32)
    hr = np.asarray(inputs["hidden_responses"], np.float32)[PERM]
    hb = np.asarray(inputs["hidden_biases"], np.float32)[PERM]
    orr = np.asarray(inputs["output_responses"], np.float32)
    ob = np.asarray(inputs["output_biases"], np.float32)

    wih_s = SCALE * (hr[:, None] *
                     np.asarray(inputs["input_to_hidden"], np.float32)[PERM]).T
    whh_s = SCALE * (hr[:, None] *
                     np.asarray(inputs["hidden_to_hidden"],
                                np.float32)[PERM][:, PERM]).T
    who_s = SCALE * (orr[:, None] *
                     np.asarray(inputs["hidden_to_output"],
                                np.float32)[:, PERM]).T
    wio_s = SCALE * (orr[:, None] *
                     np.asarray(inputs["input_to_output"], np.float32)).T

    fp8 = ml_dtypes.float8_e4m3
    # wih m-block-major DR: [p, ((b*2+t)*2+mloc)*2+i, j]
    #   = wih_s[(2t+i)*128+p, (2b+mloc)*128+j]
    wih_p = np.ascontiguousarray(
        wih_s.reshape(2, 2, 128, 8, 2, 128).transpose(2, 3, 0, 4, 1, 5)
        .reshape(128, 64, 128)).astype(fp8)

    shared = {
        "wih": wih_p,
        "whh": _pack_dr(whh_s, KP, KH),
        "who": _pack_dr(who_s, KP, KO),
        "wio": _pack_dr(wio_s, 2, KO),
        "hbc": np.ascontiguousarray(SCALE * hb.reshape(KH, 128).T),
        "obc": np.ascontiguousarray(ob.reshape(KO, 128).T),
        "bmask": _make_bmask(),
    }
    in_maps = []
    for c in range(N_CORES):
        m = dict(shared)
        # x chunk-major DR: [p, (ch*2+t)*2+i, col]
        #   = x.T[(2t+i)*128+p, ch*CH+col]
        xc = np.ascontiguousarray(x[c * BL:(c + 1) * BL].T)     # (NI, BL)
        m["xT"] = np.ascontiguousarray(
            xc.reshape(2, 2, 128, NCH, CH).transpose(2, 3, 0, 1, 4)
            .reshape(128, NCH * 4, CH)).astype(fp8)
        in_maps.append(m)
    return in_maps


def _run(inputs, trace=False, tmpdir=None):
    nc = _get_nc()
    in_maps = _prep_in_maps(inputs)
    res = run_bass_kernel_spmd(nc, in_maps, core_ids=list(range(N_CORES)),
                               trace=trace, tmpdir=tmpdir)
    out = np.empty((B, NO), np.float32)
    for c in range(N_CORES):
        out[c * BL:(c + 1) * BL] = res.results[c]["outT"].T.astype(np.float32)
    return out, res


def kernel(**inputs) -> np.ndarray:
    out, _ = _run(inputs, trace=False)
    return out


if __name__ == "__main__":
    rng = np.random.default_rng(0)
    ins = {
        "inputs": rng.standard_normal((B, NI), dtype=np.float32),
        "input_to_hidden": rng.standard_normal((NH, NI), dtype=np.float32) * 0.02,
        "hidden_to_hidden": rng.standard_normal((NH, NH), dtype=np.float32) * 0.02,
        "output_to_hidden": rng.standard_normal((NH, NO), dtype=np.float32) * 0.02,
        "input_to_output": rng.standard_normal((NO, NI), dtype=np.float32) * 0.02,
        "hidden_to_output": rng.standard_normal((NO, NH), dtype=np.float32) * 0.02,
        "output_to_output": rng.standard_normal((NO, NO), dtype=np.float32) * 0.02,
        "hidden_responses": rng.standard_normal(NH, dtype=np.float32) * 0.1 + 1.0,
        "hidden_biases": rng.standard_normal(NH, dtype=np.float32) * 0.1,
        "output_responses": rng.standard_normal(NO, dtype=np.float32) * 0.1 + 1.0,
        "output_biases": rng.standard_normal(NO, dtype=np.float32) * 0.1,
    }
    out = kernel(**ins)
    print("kernel output", out.shape, out.dtype, out[:2, :4])
